# revision 1
# baseline (speedup 1.0000x reference)
"""Trainium2 Bass kernel for nn_EntropyLM (wavelet-coeff mixer + chunked MHA + output proj).

Strategy: data-parallel over the 16 independent (batch x chunk) blocks, 2 per
NeuronCore.  All matmuls run in bf16 on the PE with fp32 PSUM accumulation;
layernorm / softmax statistics are computed in fp32.

Layout convention per chunk (CHUNK=1024 tokens, H=1024 features):
  * Linear layers contract over features, so the activation operand of each
    matmul must be feature-major ("T" tensors: [feat_part, token_free]).
  * LN / softmax reductions run along the free axis, so those stages use
    token-major tensors ([token_part, feat_free]).
  * Attention scores are computed directly transposed (ST = K @ Q^T, i.e.
    [k_part, q_free]); exp(ST) is then exactly the lhsT operand that the
    PV matmul needs, which avoids any on-chip transpose of the score matrix.
    The softmax denominator is computed with a ones-vector matmul (partition
    reduction on the PE) and applied per-partition after PV.
  * Orientation changes of bf16 activations go through the DMA xbar
    transpose engine (dma_start_transpose), never through the PE.
"""

import numpy as np
import ml_dtypes

B, S, H, G, W = 4, 4096, 1024, 256, 8
CHUNK = 1024
NUM_HEADS = 4
HD = H // NUM_HEADS          # 256 per-head dim
HM = H // 2                  # 512 mixer hidden
N_CHUNKS = B * (S // CHUNK)  # 16 independent chunks
N_CORES = 8
CPC = N_CHUNKS // N_CORES    # 2 chunks per core
NT = CHUNK // 128            # 8 token tiles
KH = H // 128                # 8 feature tiles (H)
KM = HM // 128               # 4 feature tiles (HM)
EPS = 1e-5
BF16 = ml_dtypes.bfloat16

_COMPILED = None


def _build(debug=False):
    import concourse.bass as bass  # noqa: F401
    import concourse.tile as tile
    from concourse import bacc, mybir

    bf = mybir.dt.bfloat16
    fp16 = mybir.dt.float16
    f32 = mybir.dt.float32
    Alu = mybir.AluOpType
    Act = mybir.ActivationFunctionType

    nc = bacc.Bacc("TRN2", target_bir_lowering=False, debug=False,
                   enable_asserts=True, num_devices=N_CORES)

    # ---- DRAM tensors (per-core views; same NEFF on all 8 cores) ----
    xt = nc.dram_tensor("xt", [CPC, H, CHUNK], bf, kind="ExternalInput")
    kernT = nc.dram_tensor("kernt", [H, W], bf, kind="ExternalInput")
    w1a = nc.dram_tensor("w1a", [W + 1, HM], bf, kind="ExternalInput")
    gln = nc.dram_tensor("gln", [128, KM], f32, kind="ExternalInput")
    bln = nc.dram_tensor("bln", [128, KM], f32, kind="ExternalInput")
    w2 = nc.dram_tensor("w2", [HM, H], bf, kind="ExternalInput")
    b2c = nc.dram_tensor("b2c", [128, KH], f32, kind="ExternalInput")
    wq = nc.dram_tensor("wq", [H, H], bf, kind="ExternalInput")
    wk = nc.dram_tensor("wk", [H, H], bf, kind="ExternalInput")
    wv = nc.dram_tensor("wv", [H, H], bf, kind="ExternalInput")
    wo = nc.dram_tensor("wo", [H, H], bf, kind="ExternalInput")
    gw = nc.dram_tensor("gw", [H, G], bf, kind="ExternalInput")
    bw = nc.dram_tensor("bw", [128, G], f32, kind="ExternalInput")
    y = nc.dram_tensor("y", [CPC, CHUNK, G], f32, kind="ExternalOutput")
    dbg = {}
    if debug:
        for nm, shp, dt in [
            ("dcoef", [W + 1, CHUNK], bf),
            ("dhidT", [128, KM, CHUNK], bf), ("dmixT", [128, KH, CHUNK], bf),
            ("dmixN", [128, NT, H], bf), ("dqT", [128, KH, CHUNK], bf),
            ("dkT", [128, KH, CHUNK], bf), ("dvN", [128, NT, H], fp16),
            ("det", [128, KH, CHUNK], fp16), ("docat", [128, NT, H], bf),
            ("dres", [128, NT, H], bf), ("dz", [128, NT, H], bf),
            ("dzT", [128, KH, CHUNK], bf), ("dsq", [128, NT], f32),
        ]:
            dbg[nm] = nc.dram_tensor(nm, shp, dt, kind="ExternalOutput")

    with tile.TileContext(nc) as tc:
        with (
            tc.tile_pool(name="wp", bufs=1) as wp,
            tc.tile_pool(name="ws", bufs=1) as ws,
            tc.tile_pool(name="sm", bufs=2) as sm,
            tc.tile_pool(name="ps", bufs=3, space="PSUM") as ps,
            tc.tile_pool(name="ps2", bufs=2, space="PSUM") as ps2,
        ):
            # ---------- persistent weights ----------
            kt_sb = wp.tile([128, KH, W], bf, tag="ktw")
            nc.sync.dma_start(kt_sb[:], kernT.ap().rearrange("(i p) w -> p i w", p=128))
            w1a_sb = wp.tile([W + 1, HM], bf, tag="w1a")
            nc.sync.dma_start(w1a_sb[:], w1a.ap())
            gln_sb = wp.tile([128, KM], f32, tag="gln")
            nc.sync.dma_start(gln_sb[:], gln.ap())
            bln_sb = wp.tile([128, KM], f32, tag="bln")
            nc.sync.dma_start(bln_sb[:], bln.ap())
            b2_sb = wp.tile([128, KH], f32, tag="b2")
            nc.sync.dma_start(b2_sb[:], b2c.ap())
            gw_sb = wp.tile([128, KH, G], bf, tag="gw")
            nc.sync.dma_start(gw_sb[:], gw.ap().rearrange("(i p) g -> p i g", p=128))
            bw_sb = wp.tile([128, G], f32, tag="bw")
            nc.sync.dma_start(bw_sb[:], bw.ap())
            ones_sb = wp.tile([128, 1], fp16, tag="ones")
            nc.vector.memset(ones_sb[:], 1.0)
            eps_sb = wp.tile([128, 1], f32, tag="eps")
            nc.vector.memset(eps_sb[:], EPS)

            def stream_w(src):
                dst = ws.tile([128, KH, H], bf, tag="wstream", bufs=2, name="wst")
                nc.sync.dma_start(dst[:], src.ap().rearrange("(i p) m -> p i m", p=128))
                return dst

            # ---------- stage 1 (both chunks up front): wavelet coeffs ----------
            # Running chunk 1's input load + tiny coeff matmuls during chunk 0's
            # mixer window removes the chunk-boundary DMA stall.
            coefs = []
            for c in range(CPC):
                xts = ws.tile([128, KH, CHUNK], bf, tag="xts_et", bufs=2)
                for ii in range(2):
                    nc.sync.dma_start(
                        xts[:, ii * 4:(ii + 1) * 4, :],
                        xt.ap()[c, ii * 512:(ii + 1) * 512, :].rearrange(
                            "(i p) t -> p i t", p=128))
                coef = ws.tile([W + 1, CHUNK], bf, tag="coef", bufs=2)
                # row W is the constant 1.0 bias row for the folded mix_b1
                nc.gpsimd.memset(coef[:, :], 1.0)
                for n in range(2):
                    cps = ps.tile([128, 512], f32, tag="mm")
                    for i in range(KH):
                        nc.tensor.matmul(cps[:W, :], kt_sb[:, i, :],
                                         xts[:, i, n * 512:(n + 1) * 512],
                                         start=(i == 0), stop=(i == KH - 1))
                    nc.scalar.copy(coef[:W, n * 512:(n + 1) * 512], cps[:W, :])
                coefs.append(coef)

            for c in range(CPC):
                coef = coefs[c]
                w2s = ws.tile([128, KM, H], bf, tag="wstream", bufs=2, name="w2s")
                nc.sync.dma_start(w2s[:], w2.ap().rearrange("(i p) m -> p i m", p=128))
                wq_sb = stream_w(wq)
                wk_sb = stream_w(wk)
                if debug and c == 0:
                    nc.sync.dma_start(dbg["dcoef"].ap(), coef[:])
                # ---------- stage 2: mixer hidden + LN + gelu -> hidT ----------
                # z1 = (pre-m)*inv in token-major (stats per-partition), then
                # transpose; gamma/beta + gelu applied feature-major where
                # they are per-partition -> one fused TS + in-place gelu.
                hidT = ws.tile([128, KM, CHUNK], bf, tag="hidT")
                for t in range(NT):
                    hps = ps.tile([128, 512], f32, tag="mm")
                    nc.tensor.matmul(hps[:], coef[:, t * 128:(t + 1) * 128],
                                     w1a_sb[:], start=True, stop=True)
                    st6 = sm.tile([128, 6], f32, tag="st6")
                    nc.vector.bn_stats(st6[:], hps[:])
                    mv = sm.tile([128, 2], f32, tag="mv")
                    nc.vector.bn_aggr(mv[:], st6[:])
                    sq = sm.tile([128, 1], f32, tag="sq")
                    nc.scalar.activation(sq[:], mv[:, 1:2], Act.Sqrt, bias=eps_sb[:])
                    iv = sm.tile([128, 1], f32, tag="iv")
                    nc.vector.reciprocal(iv[:], sq[:])
                    tmp = sm.tile([128, HM], bf, tag="mtmp")
                    nc.vector.tensor_scalar(tmp[:], hps[:],
                                            mv[:, 0:1], iv[:],
                                            op0=Alu.subtract, op1=Alu.mult)
                    nc.sync.dma_start_transpose(hidT[:, :, t * 128:(t + 1) * 128],
                                                tmp[:])
                for nh in range(2):
                    for ki in range(KM):
                        sl = hidT[:, ki, nh * 512:(nh + 1) * 512]
                        nc.vector.tensor_scalar(sl, sl,
                                                gln_sb[:, ki:ki + 1], bln_sb[:, ki:ki + 1],
                                                op0=Alu.mult, op1=Alu.add)
                        nc.scalar.activation(sl, sl, Act.Gelu)

                if debug and c == 0:
                    nc.sync.dma_start(dbg["dhidT"].ap(), hidT[:])
                # ---------- stage 3: mixedT (+b2) and mixed_nat ----------
                mixT = ws.tile([128, KH, CHUNK], bf, tag="mixT_z", bufs=2)
                for n in range(2):
                    for m in range(KH):
                        mps = ps.tile([128, 512], f32, tag="mm")
                        for ki in range(KM):
                            nc.tensor.matmul(mps[:], w2s[:, ki, m * 128:(m + 1) * 128],
                                             hidT[:, ki, n * 512:(n + 1) * 512],
                                             start=(ki == 0), stop=(ki == KM - 1))
                        nc.vector.tensor_scalar(mixT[:, m, n * 512:(n + 1) * 512], mps[:],
                                                b2_sb[:, m:m + 1], None, op0=Alu.add)
                mixN = ws.tile([128, NT, H], bf, tag="mixN")
                for m in range(KH):
                    nc.sync.dma_start_transpose(mixN[:, :, m * 128:(m + 1) * 128],
                                                mixT[:, m, :])

                if debug and c == 0:
                    nc.sync.dma_start(dbg["dmixT"].ap(), mixT[:])
                    nc.sync.dma_start(dbg["dmixN"].ap(), mixN[:])
                # ---------- stage 4: qT, kT, v ----------
                qT = ws.tile([128, KH, CHUNK], bf, tag="qT_otc")
                kT = ws.tile([128, KH, CHUNK], bf, tag="kT_zT")
                for (dst, wsb, on_act) in ((qT, wq_sb, True), (kT, wk_sb, False)):
                    for n in range(2):
                        for m in range(KH):
                            qps = ps.tile([128, 512], f32, tag="mm")
                            for ki in range(KH):
                                nc.tensor.matmul(qps[:], wsb[:, ki, m * 128:(m + 1) * 128],
                                                 mixT[:, ki, n * 512:(n + 1) * 512],
                                                 start=(ki == 0), stop=(ki == KH - 1))
                            if on_act:
                                nc.scalar.copy(dst[:, m, n * 512:(n + 1) * 512], qps[:])
                            else:
                                nc.vector.tensor_copy(dst[:, m, n * 512:(n + 1) * 512], qps[:])
                wv_sb = stream_w(wv)
                vN = ws.tile([128, NT, H], fp16, tag="hp_v")
                for t in range(NT):
                    for n in range(2):
                        vps = ps.tile([128, 512], f32, tag="mm")
                        for ki in range(KH):
                            nc.tensor.matmul(vps[:], mixT[:, ki, t * 128:(t + 1) * 128],
                                             wv_sb[:, ki, n * 512:(n + 1) * 512],
                                             start=(ki == 0), stop=(ki == KH - 1))
                        nc.scalar.copy(vN[:, t, n * 512:(n + 1) * 512], vps[:])

                if debug and c == 0:
                    nc.sync.dma_start(dbg["dqT"].ap(), qT[:])
                    nc.sync.dma_start(dbg["dkT"].ap(), kT[:])
                    nc.sync.dma_start(dbg["dvN"].ap(), vN[:])
                wo_sb = stream_w(wo)
                # ---------- stage 5: attention ----------
                ocat = ws.tile([128, NT, H], bf, tag="hidT_oc_res")
                if debug and c == 0:
                    dsq_sb = sm.tile([128, NT], f32, tag="dsq")
                for h in range(NUM_HEADS):
                    et = ws.tile([128, KH, CHUNK], fp16, tag="xts_et", bufs=2)
                    for kt in range(NT):
                        stp = ps2.tile([128, CHUNK], f32, tag="st")
                        for qn in range(2):
                            for dk in range(2):
                                nc.tensor.matmul(
                                    stp[:, qn * 512:(qn + 1) * 512],
                                    kT[:, 2 * h + dk, kt * 128:(kt + 1) * 128],
                                    qT[:, 2 * h + dk, qn * 512:(qn + 1) * 512],
                                    start=(dk == 0), stop=(dk == 1))
                        # exp(score/sqrt(hd)); values are O(1e-1) so no max-sub needed
                        nc.scalar.activation(et[:, kt, :], stp[:], Act.Exp,
                                             scale=float(HD ** -0.5))
                    for qt in range(NT):
                        ovp = ps.tile([128, 512], f32, tag="mm")
                        for kt in range(NT):
                            # O_unnorm[q, d] accumulation; the extra N=1 matmul
                            # with a ones column gives s[q] = sum_k exp in the
                            # same [q_part, 1] orientation the normalization
                            # needs (same lhsT -> weight load is reused).
                            nc.tensor.matmul(ovp[:, :HD], et[:, kt, qt * 128:(qt + 1) * 128],
                                             vN[:, kt, h * HD:(h + 1) * HD],
                                             start=(kt == 0), stop=(kt == NT - 1))
                            # start=False even at kt==0: start=True clears the
                            # whole PSUM bank and would wipe the V-matmul's
                            # kt==0 contribution.  The bank-clear from the
                            # V-matmul above leaves this column's has_written
                            # bits 0, so kt==0 overwrites (not accumulates).
                            nc.tensor.matmul(ovp[:, HD:HD + 1],
                                             et[:, kt, qt * 128:(qt + 1) * 128],
                                             ones_sb[:],
                                             start=False, stop=(kt == NT - 1),
                                             skip_group_check=True)
                        rq = sm.tile([128, 1], f32, tag="rq")
                        if debug and c == 0 and h == NUM_HEADS - 1:
                            nc.vector.tensor_copy(dsq_sb[:, qt:qt + 1], ovp[:, HD:HD + 1])
                        nc.vector.reciprocal(rq[:], ovp[:, HD:HD + 1])
                        nc.vector.tensor_scalar(ocat[:, qt, h * HD:(h + 1) * HD],
                                                ovp[:, :HD], rq[:], None,
                                                op0=Alu.mult)
                otc = ws.tile([128, KH, CHUNK], bf, tag="qT_otc")
                for qt in range(NT):
                    nc.sync.dma_start_transpose(otc[:, :, qt * 128:(qt + 1) * 128],
                                                ocat[:, qt, :])

                if debug and c == 0:
                    nc.sync.dma_start(dbg["det"].ap(), et[:])
                    nc.sync.dma_start(dbg["docat"].ap(), ocat[:])
                    nc.sync.dma_start(dbg["dsq"].ap(), dsq_sb[:])
                # ---------- stage 6: wo proj + residual + out LN ----------
                res = ws.tile([128, NT, H], bf, tag="hidT_oc_res")
                z = ws.tile([128, NT, H], bf, tag="mixT_z", bufs=2)
                zT = ws.tile([128, KH, CHUNK], bf, tag="kT_zT")
                for t in range(NT):
                    for n in range(2):
                        ops_ = ps.tile([128, 512], f32, tag="mm")
                        for fi in range(KH):
                            nc.tensor.matmul(ops_[:], otc[:, fi, t * 128:(t + 1) * 128],
                                             wo_sb[:, fi, n * 512:(n + 1) * 512],
                                             start=(fi == 0), stop=(fi == KH - 1))
                        nc.vector.tensor_add(res[:, t, n * 512:(n + 1) * 512], ops_[:],
                                             mixN[:, t, n * 512:(n + 1) * 512])
                    st6 = sm.tile([128, 2, 6], f32, tag="st6b")
                    for half in range(2):
                        nc.vector.bn_stats(st6[:, half, :],
                                           res[:, t, half * 512:(half + 1) * 512])
                    mv = sm.tile([128, 2], f32, tag="mv")
                    nc.vector.bn_aggr(mv[:], st6[:])
                    sq = sm.tile([128, 1], f32, tag="sq")
                    nc.scalar.activation(sq[:], mv[:, 1:2], Act.Sqrt, bias=eps_sb[:])
                    iv = sm.tile([128, 1], f32, tag="iv")
                    nc.vector.reciprocal(iv[:], sq[:])
                    nc.vector.tensor_scalar(z[:, t, :], res[:, t, :],
                                            mv[:, 0:1], iv[:],
                                            op0=Alu.subtract, op1=Alu.mult)
                    nc.sync.dma_start_transpose(zT[:, :, t * 128:(t + 1) * 128],
                                                z[:, t, :])

                if debug and c == 0:
                    nc.sync.dma_start(dbg["dres"].ap(), res[:])
                    nc.sync.dma_start(dbg["dz"].ap(), z[:])
                    nc.sync.dma_start(dbg["dzT"].ap(), zT[:])
                # ---------- stage 7: output projection ----------
                ych = ws.tile([128, NT, G], f32, tag="ych", bufs=1)
                for t in range(NT):
                    yps = ps.tile([128, 512], f32, tag="mm")
                    for fi in range(KH):
                        nc.tensor.matmul(yps[:, :G], zT[:, fi, t * 128:(t + 1) * 128],
                                         gw_sb[:, fi, :],
                                         start=(fi == 0), stop=(fi == KH - 1))
                    nc.vector.tensor_add(ych[:, t, :], yps[:, :G], bw_sb[:])
                for hh in range(2):
                    nc.sync.dma_start(
                        y.ap()[c, hh * 512:(hh + 1) * 512, :].rearrange(
                            "(t p) g -> p t g", p=128),
                        ych[:, hh * 4:(hh + 1) * 4, :])

    nc.compile()
    return nc


def _get_compiled():
    global _COMPILED
    if _COMPILED is None:
        _COMPILED = _build()
    return _COMPILED


def _prep_inputs(inputs):
    f32 = np.float32

    def a(name):
        return np.asarray(inputs[name], dtype=f32)

    x = a("x")
    mw = a("mother_wavelets")
    scales = a("scales")
    norm = np.sqrt(np.sum(mw ** 2, axis=2, keepdims=True))
    kern = (mw / np.maximum(norm, 1e-12)) * (1.0 / (1.0 + np.exp(-scales)))
    kern = kern[0, :, :, 0]                      # (W, H)
    kernT = np.ascontiguousarray(kern.T).astype(BF16)

    w1a = np.concatenate([a("mix_w1"), a("mix_b1")[None, :]], axis=0).astype(BF16)
    gln = np.ascontiguousarray(a("mix_ln_g").reshape(KM, 128).T).astype(f32)
    bln = np.ascontiguousarray(a("mix_ln_b").reshape(KM, 128).T).astype(f32)
    w2 = a("mix_w2").astype(BF16)
    b2c = np.ascontiguousarray(a("mix_b2").reshape(KH, 128).T).astype(f32)
    gw = (a("out_ln_g")[:, None] * a("out_w")).astype(BF16)
    bw_vec = a("out_ln_b") @ a("out_w") + a("out_b")
    bw = np.tile(bw_vec[None, :], (128, 1)).astype(f32)

    shared = {
        "kernt": kernT, "w1a": w1a, "gln": gln, "bln": bln, "w2": w2,
        "b2c": b2c, "wq": a("wq").astype(BF16), "wk": a("wk").astype(BF16),
        "wv": a("wv").astype(BF16), "wo": a("wo").astype(BF16),
        "gw": gw, "bw": bw,
    }

    xc = x.reshape(N_CHUNKS, CHUNK, H)
    xt_all = np.ascontiguousarray(xc.transpose(0, 2, 1)).astype(BF16)  # (16, H, CHUNK)
    in_maps = []
    for core in range(N_CORES):
        m = dict(shared)
        m["xt"] = np.ascontiguousarray(xt_all[core * CPC:(core + 1) * CPC])
        in_maps.append(m)
    return in_maps


def kernel(**inputs) -> np.ndarray:
    from concourse.bass_utils import run_bass_kernel_spmd

    nc = _get_compiled()
    in_maps = _prep_inputs(inputs)
    res = run_bass_kernel_spmd(nc, in_maps, core_ids=list(range(N_CORES)))
    out = np.concatenate([r["y"] for r in res.results], axis=0)  # (16, CHUNK, G)
    return out.reshape(B, S, G).astype(np.float32)



# revision 3
# speedup vs baseline: 1.3694x; 1.3694x over previous
"""Trainium2 Bass kernel for nn_EntropyLM (wavelet-coeff mixer + chunked MHA + output proj).

Strategy: data-parallel over the 16 independent (batch x chunk) blocks, 2 per
NeuronCore.  The error-critical path (wavelet coeffs, mixer, output proj and
all 16-bit storage) runs in fp16 (same PE cost as bf16, 7.5x less rounding
error).  The attention path (q/k/v projections, scores, PV, wo) runs in
fp8e4m3 with DoubleRow matmuls: contraction pairs are packed into the PE's
double-pumped fp8 mode, which both halves instruction count and doubles
throughput.  Attention tolerates fp8 because the scores here are tiny
(std ~0.04), softmax is near-uniform, and the whole attention output is a
small additive correction to the residual stream.

All fp8 tensors are pre-scaled by exact powers of two (weights x16,
mixT x8, q/k/v x4, ocat x16) to lift values out of e4m3's subnormal range;
compensating 2^-k factors fold into existing evacuation instructions
(Act scale immediates / tensor_scalar scalars), so the rescaling is free.

Layout convention per chunk (CHUNK=1024 tokens, H=1024 features):
  * Linear layers contract over features -> feature-major operands
    ("T" tensors: [feat_part, token_free]); LN / softmax statistics run
    token-major.  DoubleRow pair slots hold consecutive 128-row contraction
    tiles: AP [128, 2, N] with pair stride = row stride of the tile.
  * Attention scores are computed directly transposed (ST = K @ Q^T,
    [k_part, q_free]); exp(ST) in fp8 is exactly the lhsT operand of the PV
    matmul.  The softmax denominator rides as a 1-column DoubleRow matmul
    against a constant 0.25 vector, reusing the et weights.
  * 16-bit orientation changes go through the DMA xbar transpose engine.
"""

import numpy as np
import ml_dtypes

B, S, H, G, W = 4, 4096, 1024, 256, 8
CHUNK = 1024
NUM_HEADS = 4
HD = H // NUM_HEADS          # 256 per-head dim
HM = H // 2                  # 512 mixer hidden
N_CHUNKS = B * (S // CHUNK)  # 16 independent chunks
N_CORES = 8
CPC = N_CHUNKS // N_CORES    # 2 chunks per core
NT = CHUNK // 128            # 8 token tiles
KH = H // 128                # 8 feature tiles (H)
KM = HM // 128               # 4 feature tiles (HM)
KP = KH // 2                 # 4 DoubleRow pair tiles (contraction H)
EPS = 1e-5
F16 = np.float16
FP8 = ml_dtypes.float8_e4m3  # IEEE e4m3 (max 240) == TRN fp8_e4m3

# power-of-2 fp8 pre-scales (all exact in fp)
WSC = 16.0                   # wq/wk/wv/wo uploaded as w*16
MSC = 8.0                    # mixT8 = mixT*8
QSC = 4.0                    # qT8/kT8/vN8 hold 4*q etc; evac scale 4/(16*8)
OSC = 16.0                   # ocat holds 16*o_norm (ones=0.25 arranges this)

_COMPILED = None


def _build(debug=False):
    import concourse.bass as bass  # noqa: F401
    import concourse.tile as tile
    from concourse import bacc, mybir

    fp16 = mybir.dt.float16
    fp8 = mybir.dt.float8e4
    f32 = mybir.dt.float32
    Alu = mybir.AluOpType
    Act = mybir.ActivationFunctionType
    DR = mybir.MatmulPerfMode.DoubleRow

    nc = bacc.Bacc("TRN2", target_bir_lowering=False, debug=False,
                   enable_asserts=True, num_devices=N_CORES)

    # ---- DRAM tensors (per-core views; same NEFF on all 8 cores) ----
    xt = nc.dram_tensor("xt", [CPC, H, CHUNK], fp16, kind="ExternalInput")
    kernT = nc.dram_tensor("kernt", [H, W], fp16, kind="ExternalInput")
    w1a = nc.dram_tensor("w1a", [W + 1, HM], fp16, kind="ExternalInput")
    gln = nc.dram_tensor("gln", [128, KM], f32, kind="ExternalInput")
    bln = nc.dram_tensor("bln", [128, KM], f32, kind="ExternalInput")
    w2 = nc.dram_tensor("w2", [HM, H], fp16, kind="ExternalInput")
    b2c = nc.dram_tensor("b2c", [128, KH], f32, kind="ExternalInput")
    wq8 = nc.dram_tensor("wq8", [H, H], fp8, kind="ExternalInput")
    wk8 = nc.dram_tensor("wk8", [H, H], fp8, kind="ExternalInput")
    wv8 = nc.dram_tensor("wv8", [H, H], fp8, kind="ExternalInput")
    wo8 = nc.dram_tensor("wo8", [H, H], fp8, kind="ExternalInput")
    gw = nc.dram_tensor("gw", [H, G], fp16, kind="ExternalInput")
    bw = nc.dram_tensor("bw", [128, G], f32, kind="ExternalInput")
    y = nc.dram_tensor("y", [CPC, CHUNK, G], f32, kind="ExternalOutput")

    with tile.TileContext(nc) as tc:
        with (
            tc.tile_pool(name="wp", bufs=1) as wp,
            tc.tile_pool(name="ws", bufs=1) as ws,
            tc.tile_pool(name="sm", bufs=2) as sm,
            tc.tile_pool(name="ps", bufs=3, space="PSUM") as ps,
            tc.tile_pool(name="ps2", bufs=2, space="PSUM") as ps2,
        ):
            # ---------- persistent weights ----------
            kt_sb = wp.tile([128, KH, W], fp16, tag="ktw")
            nc.sync.dma_start(kt_sb[:], kernT.ap().rearrange("(i p) w -> p i w", p=128))
            w1a_sb = wp.tile([W + 1, HM], fp16, tag="w1a")
            nc.sync.dma_start(w1a_sb[:], w1a.ap())
            gln_sb = wp.tile([128, KM], f32, tag="gln")
            nc.sync.dma_start(gln_sb[:], gln.ap())
            bln_sb = wp.tile([128, KM], f32, tag="bln")
            nc.sync.dma_start(bln_sb[:], bln.ap())
            b2_sb = wp.tile([128, KH], f32, tag="b2")
            nc.sync.dma_start(b2_sb[:], b2c.ap())
            gw_sb = wp.tile([128, KH, G], fp16, tag="gw")
            nc.sync.dma_start(gw_sb[:], gw.ap().rearrange("(i p) g -> p i g", p=128))
            bw_sb = wp.tile([128, G], f32, tag="bw")
            nc.sync.dma_start(bw_sb[:], bw.ap())
            ones_sb = wp.tile([128, 2, 1], fp8, tag="ones")
            nc.vector.memset(ones_sb[:], 0.25)
            eps_sb = wp.tile([128, 1], f32, tag="eps")
            nc.vector.memset(eps_sb[:], EPS)
            # fp8 weights resident across both chunks
            wq_sb = wp.tile([128, KH, H], fp8, tag="wq8")
            nc.sync.dma_start(wq_sb[:], wq8.ap().rearrange("(i p) m -> p i m", p=128))
            wk_sb = wp.tile([128, KH, H], fp8, tag="wk8")
            nc.sync.dma_start(wk_sb[:], wk8.ap().rearrange("(i p) m -> p i m", p=128))
            wv_sb = wp.tile([128, KH, H], fp8, tag="wv8")
            nc.sync.dma_start(wv_sb[:], wv8.ap().rearrange("(i p) m -> p i m", p=128))
            wo_sb = wp.tile([128, KH, H], fp8, tag="wo8")
            nc.sync.dma_start(wo_sb[:], wo8.ap().rearrange("(i p) m -> p i m", p=128))
            w2_sb = wp.tile([128, KM, H], fp16, tag="w2")
            nc.sync.dma_start(w2_sb[:], w2.ap().rearrange("(i p) m -> p i m", p=128))

            # ---------- stage 1 (both chunks up front): wavelet coeffs ----------
            coefs = []
            for c in range(CPC):
                xts = ws.tile([128, KH, CHUNK], fp16, tag="xts_et", bufs=2)
                for ii in range(2):
                    nc.sync.dma_start(
                        xts[:, ii * 4:(ii + 1) * 4, :],
                        xt.ap()[c, ii * 512:(ii + 1) * 512, :].rearrange(
                            "(i p) t -> p i t", p=128))
                coef = ws.tile([W + 1, CHUNK], fp16, tag="coef", bufs=2)
                # row W is the constant 1.0 bias row for the folded mix_b1
                nc.gpsimd.memset(coef[:, :], 1.0)
                for n in range(2):
                    cps = ps.tile([128, 512], f32, tag="mm")
                    for i in range(KH):
                        nc.tensor.matmul(cps[:W, :], kt_sb[:, i, :],
                                         xts[:, i, n * 512:(n + 1) * 512],
                                         start=(i == 0), stop=(i == KH - 1))
                    nc.vector.tensor_copy(coef[:W, n * 512:(n + 1) * 512], cps[:W, :])
                coefs.append(coef)

            for c in range(CPC):
                coef = coefs[c]
                # ---------- stage 2: mixer hidden + LN + gelu -> hidT ----------
                hidT = ws.tile([128, KM, CHUNK], fp16, tag="hidT")
                for t in range(NT):
                    hps = ps.tile([128, 512], f32, tag="mm")
                    nc.tensor.matmul(hps[:], coef[:, t * 128:(t + 1) * 128],
                                     w1a_sb[:], start=True, stop=True)
                    st6 = sm.tile([128, 6], f32, tag="st6")
                    nc.vector.bn_stats(st6[:], hps[:])
                    mv = sm.tile([128, 2], f32, tag="mv")
                    nc.vector.bn_aggr(mv[:], st6[:])
                    sq = sm.tile([128, 1], f32, tag="sq")
                    nc.scalar.activation(sq[:], mv[:, 1:2], Act.Sqrt, bias=eps_sb[:])
                    iv = sm.tile([128, 1], f32, tag="iv")
                    nc.vector.reciprocal(iv[:], sq[:])
                    tmp = sm.tile([128, HM], fp16, tag="mtmp")
                    nc.vector.tensor_scalar(tmp[:], hps[:],
                                            mv[:, 0:1], iv[:],
                                            op0=Alu.subtract, op1=Alu.mult)
                    nc.sync.dma_start_transpose(hidT[:, :, t * 128:(t + 1) * 128],
                                                tmp[:])
                # gamma/beta (per-partition feature-major) + gelu
                for nh in range(2):
                    for ki in range(KM):
                        sl = hidT[:, ki, nh * 512:(nh + 1) * 512]
                        nc.gpsimd.tensor_scalar(sl, sl,
                                                gln_sb[:, ki:ki + 1], bln_sb[:, ki:ki + 1],
                                                op0=Alu.mult, op1=Alu.add)
                        nc.scalar.activation(sl, sl, Act.Gelu)

                # ---------- stage 3: mixedT (+b2), mixT8, mixed_nat ----------
                mixT = ws.tile([128, KH, CHUNK], fp16, tag="mixT_z_otc", bufs=2)
                mixT8 = ws.tile([128, KH, CHUNK], fp8, tag="mix8")
                for n in range(2):
                    for m in range(KH):
                        mps = ps.tile([128, 512], f32, tag="mm")
                        for ki in range(KM):
                            nc.tensor.matmul(mps[:], w2_sb[:, ki, m * 128:(m + 1) * 128],
                                             hidT[:, ki, n * 512:(n + 1) * 512],
                                             start=(ki == 0), stop=(ki == KM - 1))
                        sl = mixT[:, m, n * 512:(n + 1) * 512]
                        nc.scalar.activation(sl, mps[:], Act.Identity,
                                             bias=b2_sb[:, m:m + 1])
                        nc.gpsimd.tensor_scalar(mixT8[:, m, n * 512:(n + 1) * 512],
                                                sl, MSC, None, op0=Alu.mult)
                mixN = ws.tile([128, NT, H], fp16, tag="mixN")
                for m in range(KH):
                    nc.sync.dma_start_transpose(mixN[:, :, m * 128:(m + 1) * 128],
                                                mixT[:, m, :])

                # ---------- stage 4: qT8, kT8 (feature-major), vN8 ----------
                qT8 = ws.tile([128, KH, CHUNK], fp8, tag="qT_otc8")
                kT8 = ws.tile([128, KH, CHUNK], fp8, tag="kT_zT")
                for (dst, wsb, on_act) in ((qT8, wq_sb, True), (kT8, wk_sb, False)):
                    for n in range(2):
                        for m in range(KH):
                            qps = ps.tile([128, 512], f32, tag="mm")
                            for i in range(KP):
                                nc.tensor.matmul(
                                    qps[:], wsb[:, 2 * i:2 * i + 2, m * 128:(m + 1) * 128],
                                    mixT8[:, 2 * i:2 * i + 2, n * 512:(n + 1) * 512],
                                    start=(i == 0), stop=(i == KP - 1), perf_mode=DR)
                            sl = dst[:, m, n * 512:(n + 1) * 512]
                            if on_act:
                                nc.scalar.activation(sl, qps[:], Act.Copy,
                                                     scale=QSC / (WSC * MSC))
                            else:
                                nc.vector.tensor_scalar(sl, qps[:], QSC / (WSC * MSC),
                                                        None, op0=Alu.mult)
                vN8 = ws.tile([128, NT, H], fp8, tag="vN8_ych")
                for t in range(NT):
                    for n in range(2):
                        vps = ps.tile([128, 512], f32, tag="mm")
                        for i in range(KP):
                            nc.tensor.matmul(
                                vps[:], mixT8[:, 2 * i:2 * i + 2, t * 128:(t + 1) * 128],
                                wv_sb[:, 2 * i:2 * i + 2, n * 512:(n + 1) * 512],
                                start=(i == 0), stop=(i == KP - 1), perf_mode=DR)
                        nc.scalar.activation(vN8[:, t, n * 512:(n + 1) * 512], vps[:],
                                             Act.Copy, scale=QSC / (WSC * MSC))

                # ---------- stage 5: attention ----------
                # scores arrive as 16*(q.k); exp folds hd^-0.5/16
                ocat = ws.tile([128, NT, H], fp16, tag="ocat_res")
                for h in range(NUM_HEADS):
                    et = ws.tile([128, KH, CHUNK], fp8, tag="xts_et", bufs=2)
                    for kt in range(NT):
                        stp = ps2.tile([128, CHUNK], f32, tag="st")
                        for qn in range(2):
                            nc.tensor.matmul(
                                stp[:, qn * 512:(qn + 1) * 512],
                                kT8[:, 2 * h:2 * h + 2, kt * 128:(kt + 1) * 128],
                                qT8[:, 2 * h:2 * h + 2, qn * 512:(qn + 1) * 512],
                                start=True, stop=True, perf_mode=DR)
                        nc.scalar.activation(et[:, kt, :], stp[:], Act.Exp,
                                             scale=float(HD ** -0.5) / (QSC * QSC))
                    for qt in range(NT):
                        ovp = ps.tile([128, 512], f32, tag="mm")
                        for i in range(KP):
                            nc.tensor.matmul(
                                ovp[:, :HD], et[:, 2 * i:2 * i + 2, qt * 128:(qt + 1) * 128],
                                vN8[:, 2 * i:2 * i + 2, h * HD:(h + 1) * HD],
                                start=(i == 0), stop=(i == KP - 1), perf_mode=DR)
                            # denominator column: same et weights, 0.25-vector.
                            # start=False even at i==0: the V-matmul's bank
                            # clear leaves has_written=0 here, so i==0
                            # overwrites rather than accumulates.
                            nc.tensor.matmul(
                                ovp[:, HD:HD + 1],
                                et[:, 2 * i:2 * i + 2, qt * 128:(qt + 1) * 128],
                                ones_sb[:],
                                start=False, stop=(i == KP - 1),
                                perf_mode=DR, skip_group_check=True)
                        rq = sm.tile([128, 1], f32, tag="rq")
                        nc.vector.reciprocal(rq[:], ovp[:, HD:HD + 1])
                        # ocat = (4*O) * 1/(s/4) = 16*o_norm
                        nc.vector.tensor_scalar(ocat[:, qt, h * HD:(h + 1) * HD],
                                                ovp[:, :HD], rq[:], None,
                                                op0=Alu.mult)
                otc16 = ws.tile([128, KH, CHUNK], fp16, tag="mixT_z_otc", bufs=2)
                for qt in range(NT):
                    nc.sync.dma_start_transpose(otc16[:, :, qt * 128:(qt + 1) * 128],
                                                ocat[:, qt, :])
                otc8 = ws.tile([128, KH, CHUNK], fp8, tag="qT_otc8")
                for fi in range(KH):
                    nc.gpsimd.tensor_copy(otc8[:, fi, :], otc16[:, fi, :])

                # ---------- stage 6: wo proj + residual + out LN ----------
                res = ws.tile([128, NT, H], fp16, tag="ocat_res")
                z = ws.tile([128, NT, H], fp16, tag="mixT_z_otc", bufs=2)
                zT = ws.tile([128, KH, CHUNK], fp16, tag="kT_zT")
                for t in range(NT):
                    for n in range(2):
                        ops_ = ps.tile([128, 512], f32, tag="mm")
                        for i in range(KP):
                            nc.tensor.matmul(
                                ops_[:], otc8[:, 2 * i:2 * i + 2, t * 128:(t + 1) * 128],
                                wo_sb[:, 2 * i:2 * i + 2, n * 512:(n + 1) * 512],
                                start=(i == 0), stop=(i == KP - 1), perf_mode=DR)
                        # res = psum/(16*16) + mixN
                        nc.vector.scalar_tensor_tensor(
                            res[:, t, n * 512:(n + 1) * 512], ops_[:],
                            1.0 / (OSC * WSC),
                            mixN[:, t, n * 512:(n + 1) * 512],
                            op0=Alu.mult, op1=Alu.add)
                    st6 = sm.tile([128, 2, 6], f32, tag="st6b")
                    for half in range(2):
                        nc.vector.bn_stats(st6[:, half, :],
                                           res[:, t, half * 512:(half + 1) * 512])
                    mv = sm.tile([128, 2], f32, tag="mv")
                    nc.vector.bn_aggr(mv[:], st6[:])
                    sq = sm.tile([128, 1], f32, tag="sq")
                    nc.scalar.activation(sq[:], mv[:, 1:2], Act.Sqrt, bias=eps_sb[:])
                    iv = sm.tile([128, 1], f32, tag="iv")
                    nc.vector.reciprocal(iv[:], sq[:])
                    nc.gpsimd.tensor_scalar(z[:, t, :], res[:, t, :],
                                            mv[:, 0:1], iv[:],
                                            op0=Alu.subtract, op1=Alu.mult)
                    nc.sync.dma_start_transpose(zT[:, :, t * 128:(t + 1) * 128],
                                                z[:, t, :])

                # ---------- stage 7: output projection ----------
                ych = ws.tile([128, NT, G], f32, tag="vN8_ych")
                for t in range(NT):
                    yps = ps.tile([128, 512], f32, tag="mm")
                    for fi in range(KH):
                        nc.tensor.matmul(yps[:, :G], zT[:, fi, t * 128:(t + 1) * 128],
                                         gw_sb[:, fi, :],
                                         start=(fi == 0), stop=(fi == KH - 1))
                    nc.vector.tensor_add(ych[:, t, :], yps[:, :G], bw_sb[:])
                for hh in range(2):
                    nc.sync.dma_start(
                        y.ap()[c, hh * 512:(hh + 1) * 512, :].rearrange(
                            "(t p) g -> p t g", p=128),
                        ych[:, hh * 4:(hh + 1) * 4, :])

    nc.compile()
    return nc


def _get_compiled():
    global _COMPILED
    if _COMPILED is None:
        _COMPILED = _build()
    return _COMPILED


def _prep_inputs(inputs):
    f32 = np.float32

    def a(name):
        return np.asarray(inputs[name], dtype=f32)

    x = a("x")
    mw = a("mother_wavelets")
    scales = a("scales")
    norm = np.sqrt(np.sum(mw ** 2, axis=2, keepdims=True))
    kern = (mw / np.maximum(norm, 1e-12)) * (1.0 / (1.0 + np.exp(-scales)))
    kern = kern[0, :, :, 0]                      # (W, H)
    kernT = np.ascontiguousarray(kern.T).astype(F16)

    w1a = np.concatenate([a("mix_w1"), a("mix_b1")[None, :]], axis=0).astype(F16)
    gln = np.ascontiguousarray(a("mix_ln_g").reshape(KM, 128).T).astype(f32)
    bln = np.ascontiguousarray(a("mix_ln_b").reshape(KM, 128).T).astype(f32)
    w2 = a("mix_w2").astype(F16)
    b2c = np.ascontiguousarray(a("mix_b2").reshape(KH, 128).T).astype(f32)
    gw = (a("out_ln_g")[:, None] * a("out_w")).astype(F16)
    bw_vec = a("out_ln_b") @ a("out_w") + a("out_b")
    bw = np.tile(bw_vec[None, :], (128, 1)).astype(f32)

    shared = {
        "kernt": kernT, "w1a": w1a, "gln": gln, "bln": bln, "w2": w2,
        "b2c": b2c,
        "wq8": (a("wq") * WSC).astype(FP8), "wk8": (a("wk") * WSC).astype(FP8),
        "wv8": (a("wv") * WSC).astype(FP8), "wo8": (a("wo") * WSC).astype(FP8),
        "gw": gw, "bw": bw,
    }

    xc = x.reshape(N_CHUNKS, CHUNK, H)
    xt_all = np.ascontiguousarray(xc.transpose(0, 2, 1)).astype(F16)  # (16, H, CHUNK)
    in_maps = []
    for core in range(N_CORES):
        m = dict(shared)
        m["xt"] = np.ascontiguousarray(xt_all[core * CPC:(core + 1) * CPC])
        in_maps.append(m)
    return in_maps


def kernel(**inputs) -> np.ndarray:
    from concourse.bass_utils import run_bass_kernel_spmd

    nc = _get_compiled()
    in_maps = _prep_inputs(inputs)
    res = run_bass_kernel_spmd(nc, in_maps, core_ids=list(range(N_CORES)))
    out = np.concatenate([r["y"] for r in res.results], axis=0)  # (16, CHUNK, G)
    return out.reshape(B, S, G).astype(np.float32)


# revision 60
# speedup vs baseline: 1.4122x; 1.0313x over previous
"""Trainium2 Bass kernel for nn_EntropyLM (wavelet-coeff mixer + chunked MHA + output proj).

Strategy: data-parallel over the 16 independent (batch x chunk) blocks, 2 per
NeuronCore.  The error-critical path (wavelet coeffs, mixer, output proj and
all 16-bit storage) runs in fp16 (same PE cost as bf16, 7.5x less rounding
error).  The attention path (q/k/v projections, scores, PV, wo) runs in
fp8e4m3 with DoubleRow matmuls: contraction pairs are packed into the PE's
double-pumped fp8 mode, which both halves instruction count and doubles
throughput.  Attention tolerates fp8 because the scores here are tiny
(std ~0.04), softmax is near-uniform, and the whole attention output is a
small additive correction to the residual stream.

All fp8 tensors are pre-scaled by exact powers of two (weights x16,
mixT x8, q/k/v x4, ocat x16) to lift values out of e4m3's subnormal range;
compensating 2^-k factors fold into existing evacuation instructions
(Act scale immediates / tensor_scalar scalars), so the rescaling is free.

Layout convention per chunk (CHUNK=1024 tokens, H=1024 features):
  * Linear layers contract over features -> feature-major operands
    ("T" tensors: [feat_part, token_free]); LN / softmax statistics run
    token-major.  DoubleRow pair slots hold consecutive 128-row contraction
    tiles: AP [128, 2, N] with pair stride = row stride of the tile.
  * Attention scores are computed directly transposed (ST = K @ Q^T,
    [k_part, q_free]); exp(ST) in fp8 is exactly the lhsT operand of the PV
    matmul.  The softmax denominator rides as a 1-column DoubleRow matmul
    against a constant 0.25 vector, reusing the et weights.
  * 16-bit orientation changes go through the DMA xbar transpose engine.
"""

import numpy as np
import ml_dtypes

B, S, H, G, W = 4, 4096, 1024, 256, 8
CHUNK = 1024
NUM_HEADS = 4
HD = H // NUM_HEADS          # 256 per-head dim
HM = H // 2                  # 512 mixer hidden
N_CHUNKS = B * (S // CHUNK)  # 16 independent chunks
N_CORES = 8
CPC = N_CHUNKS // N_CORES    # 2 chunks per core
NT = CHUNK // 128            # 8 token tiles
KH = H // 128                # 8 feature tiles (H)
KM = HM // 128               # 4 feature tiles (HM)
KP = KH // 2                 # 4 DoubleRow pair tiles (contraction H)
EPS = 1e-5
F16 = np.float16
FP8 = ml_dtypes.float8_e4m3  # IEEE e4m3 (max 240) == TRN fp8_e4m3

# power-of-2 fp8 pre-scales (all exact in fp)
WSC = 16.0                   # wq/wk/wv/wo uploaded as w*16
MSC = 8.0                    # mixT8 = mixT*8
QSC = 4.0                    # qT8/kT8/vN8 hold 4*q etc; evac scale 4/(16*8)
OSC = 16.0                   # ocat holds 16*o_norm (ones=0.25 arranges this)

_COMPILED = None


def _build(debug=False):
    import concourse.bass as bass  # noqa: F401
    import concourse.tile as tile
    from concourse import bacc, mybir

    fp16 = mybir.dt.float16
    fp8 = mybir.dt.float8e4
    f32 = mybir.dt.float32
    Alu = mybir.AluOpType
    Act = mybir.ActivationFunctionType
    DR = mybir.MatmulPerfMode.DoubleRow

    nc = bacc.Bacc("TRN2", target_bir_lowering=False, debug=False,
                   enable_asserts=True, num_devices=N_CORES)

    # ---- DRAM tensors (per-core views; same NEFF on all 8 cores) ----
    xt = nc.dram_tensor("xt", [CPC, H, CHUNK], fp16, kind="ExternalInput")
    kernT = nc.dram_tensor("kernt", [H, W], fp16, kind="ExternalInput")
    w1a = nc.dram_tensor("w1a", [W + 1, HM], fp16, kind="ExternalInput")
    gln = nc.dram_tensor("gln", [128, KM], f32, kind="ExternalInput")
    bln = nc.dram_tensor("bln", [128, KM], f32, kind="ExternalInput")
    w2 = nc.dram_tensor("w2", [HM, H], fp16, kind="ExternalInput")
    b2c = nc.dram_tensor("b2c", [128, KH], f32, kind="ExternalInput")
    wq8 = nc.dram_tensor("wq8", [H, H], fp8, kind="ExternalInput")
    wk8 = nc.dram_tensor("wk8", [H, H], fp8, kind="ExternalInput")
    wv8 = nc.dram_tensor("wv8", [H, H], fp8, kind="ExternalInput")
    wo8 = nc.dram_tensor("wo8", [H, H], fp8, kind="ExternalInput")
    gw = nc.dram_tensor("gw", [H, G], fp16, kind="ExternalInput")
    bw = nc.dram_tensor("bw", [128, G], f32, kind="ExternalInput")
    y = nc.dram_tensor("y", [CPC, CHUNK, G], f32, kind="ExternalOutput")

    with tile.TileContext(nc) as tc:
        with (
            tc.tile_pool(name="wp", bufs=1) as wp,
            tc.tile_pool(name="ws", bufs=1) as ws,
            tc.tile_pool(name="sm", bufs=2) as sm,
            tc.tile_pool(name="ps", bufs=4, space="PSUM") as ps,
            tc.tile_pool(name="ps2", bufs=2, space="PSUM") as ps2,
        ):
            # ---------- input x first (so coeffs can start before the big
            # weight DMAs drain), then persistent weights ----------
            kt_sb = wp.tile([128, KH, W], fp16, tag="ktw")
            nc.sync.dma_start(kt_sb[:], kernT.ap().rearrange("(i p) w -> p i w", p=128))
            xts_all = []
            for c in range(CPC):
                xts = ws.tile([128, KH, CHUNK], fp16, tag="xts_et", bufs=2)
                for ii in range(2):
                    nc.sync.dma_start(
                        xts[:, ii * 4:(ii + 1) * 4, :],
                        xt.ap()[c, ii * 512:(ii + 1) * 512, :].rearrange(
                            "(i p) t -> p i t", p=128))
                xts_all.append(xts)
            w1a_sb = wp.tile([W + 1, HM], fp16, tag="w1a")
            nc.sync.dma_start(w1a_sb[:], w1a.ap())
            gln_sb = wp.tile([128, KM], f32, tag="gln")
            nc.sync.dma_start(gln_sb[:], gln.ap())
            bln_sb = wp.tile([128, KM], f32, tag="bln")
            nc.sync.dma_start(bln_sb[:], bln.ap())
            b2_sb = wp.tile([128, KH], f32, tag="b2")
            nc.sync.dma_start(b2_sb[:], b2c.ap())
            gw_sb = wp.tile([128, KH, G], fp16, tag="gw")
            nc.sync.dma_start(gw_sb[:], gw.ap().rearrange("(i p) g -> p i g", p=128))
            bw2_sb = wp.tile([128, 2, G], f32, tag="bw")
            nc.sync.dma_start(bw2_sb[:, 0, :], bw.ap())
            nc.sync.dma_start(bw2_sb[:, 1, :], bw.ap())
            ones_sb = wp.tile([128, 2, 1], fp8, tag="ones")
            nc.vector.memset(ones_sb[:], 0.25)
            eps_sb = wp.tile([128, 1], f32, tag="eps")
            nc.vector.memset(eps_sb[:], EPS)
            # fp8 weights resident across both chunks
            wq_sb = wp.tile([128, KH, H], fp8, tag="wq8")
            nc.sync.dma_start(wq_sb[:], wq8.ap().rearrange("(i p) m -> p i m", p=128))
            wk_sb = wp.tile([128, KH, H], fp8, tag="wk8")
            nc.sync.dma_start(wk_sb[:], wk8.ap().rearrange("(i p) m -> p i m", p=128))
            wv_sb = wp.tile([128, KH, H], fp8, tag="wv8")
            nc.sync.dma_start(wv_sb[:], wv8.ap().rearrange("(i p) m -> p i m", p=128))
            wo_sb = wp.tile([128, KH, H], fp8, tag="wo8")
            nc.sync.dma_start(wo_sb[:], wo8.ap().rearrange("(i p) m -> p i m", p=128))
            w2_sb = wp.tile([128, KM, H], fp16, tag="w2")
            nc.sync.dma_start(w2_sb[:], w2.ap().rearrange("(i p) m -> p i m", p=128))

            # ---------- stage 1 (both chunks up front): wavelet coeffs ----------
            coefs = []
            for c in range(CPC):
                xts = xts_all[c]
                coef = ws.tile([W + 1, CHUNK], fp16, tag="coef", bufs=2)
                # row W is the constant 1.0 bias row for the folded mix_b1
                nc.gpsimd.memset(coef[:, :], 1.0)
                for n in range(2):
                    cps = ps.tile([128, 512], f32, tag="mm")
                    for i in range(KH):
                        nc.tensor.matmul(cps[:W, :], kt_sb[:, i, :],
                                         xts[:, i, n * 512:(n + 1) * 512],
                                         start=(i == 0), stop=(i == KH - 1))
                    nc.vector.tensor_copy(coef[:W, n * 512:(n + 1) * 512], cps[:W, :])
                coefs.append(coef)

            for c in range(CPC):
                coef = coefs[c]
                # ---------- stage 2: mixer hidden + LN + gelu -> hidT ----------
                # sqrt is batched per 4 tiles (one table-friendly Act op);
                # the normalize itself runs on Act as Identity(iv*x - m*iv).
                hidT = ws.tile([128, KM, CHUNK], fp16, tag="hidT_qT_otc8", bufs=2)
                hpss = {}
                mva = sm.tile([128, NT, 2], f32, tag="mva")
                iva = sm.tile([128, NT], f32, tag="iva")
                for t in range(NT):
                    hps = ps.tile([128, 512], f32, tag="mm")
                    nc.tensor.matmul(hps[:], coef[:, t * 128:(t + 1) * 128],
                                     w1a_sb[:], start=True, stop=True)
                    hpss[t] = hps
                    st6 = sm.tile([128, 6], f32, tag="st6")
                    nc.vector.bn_stats(st6[:], hps[:])
                    nc.vector.bn_aggr(mva[:, t, :], st6[:])
                    if t % 2 == 1:
                        g = t // 2
                        u2 = sm.tile([128, 2], f32, tag="u2")
                        nc.scalar.activation(u2[:], mva[:, g * 2:(g + 1) * 2, 1:2],
                                             Act.Sqrt, bias=eps_sb[:])
                        nc.vector.reciprocal(iva[:, g * 2:(g + 1) * 2], u2[:])
                        for tt in (t - 1, t):
                            tmp = sm.tile([128, HM], fp16, tag="mtmp")
                            nc.vector.tensor_scalar(tmp[:], hpss.pop(tt)[:],
                                                    mva[:, tt, 0:1],
                                                    iva[:, tt:tt + 1],
                                                    op0=Alu.subtract, op1=Alu.mult)
                            nc.sync.dma_start_transpose(
                                hidT[:, :, tt * 128:(tt + 1) * 128], tmp[:])
                # gamma/beta (per-partition feature-major) fused into gelu:
                # gelu(g*x + b) in one Act pass
                for nh in range(2):
                    for ki in range(KM):
                        sl = hidT[:, ki, nh * 512:(nh + 1) * 512]
                        nc.scalar.activation(sl, sl, Act.Gelu,
                                             scale=gln_sb[:, ki:ki + 1],
                                             bias=bln_sb[:, ki:ki + 1])

                # ---------- stage 3: mixedT (+b2), mixT8, mixed_nat ----------
                mixT = ws.tile([128, KH, CHUNK], fp16, tag="mixT_otc", bufs=2)
                mixT8 = ws.tile([128, KH, CHUNK], fp8, tag="mix8")
                # n-major so the n=0 half's matmuls can start as soon as the
                # first half of stage 2's transposes land (fills the S2 bubble)
                for n in range(2):
                    for m in range(KH):
                        mps = ps.tile([128, 512], f32, tag="mm")
                        for ki in range(KM):
                            nc.tensor.matmul(mps[:],
                                             w2_sb[:, ki, m * 128:(m + 1) * 128],
                                             hidT[:, ki, n * 512:(n + 1) * 512],
                                             start=(ki == 0), stop=(ki == KM - 1))
                        sl = mixT[:, m, n * 512:(n + 1) * 512]
                        nc.scalar.activation(sl, mps[:], Act.Identity,
                                             bias=b2_sb[:, m:m + 1])
                        nc.gpsimd.tensor_scalar(mixT8[:, m, n * 512:(n + 1) * 512],
                                                sl, MSC, None, op0=Alu.mult)
                mixN = ws.tile([128, NT, H], fp16, tag="mixN")
                for m in range(KH):
                    nc.sync.dma_start_transpose(mixN[:, :, m * 128:(m + 1) * 128],
                                                mixT[:, m, :])

                # ---------- stage 4: qT8, kT8 (feature-major), vN8 ----------
                qT8 = ws.tile([128, KH, CHUNK], fp8, tag="hidT_qT_otc8", bufs=2)
                kT8 = ws.tile([128, KH, CHUNK], fp8, tag="kT_zT")
                for (dst, wsb, on_act) in ((qT8, wq_sb, True), (kT8, wk_sb, False)):
                    for m in range(KH):
                        qps = ps2.tile([128, CHUNK], f32, tag="big")
                        for n in range(2):
                            for i in range(KP):
                                nc.tensor.matmul(
                                    qps[:, n * 512:(n + 1) * 512],
                                    wsb[:, 2 * i:2 * i + 2, m * 128:(m + 1) * 128],
                                    mixT8[:, 2 * i:2 * i + 2, n * 512:(n + 1) * 512],
                                    start=(i == 0), stop=(i == KP - 1), perf_mode=DR)
                        if on_act:
                            nc.scalar.activation(dst[:, m, :], qps[:], Act.Copy,
                                                 scale=QSC / (WSC * MSC))
                        else:
                            nc.vector.tensor_scalar(dst[:, m, :], qps[:],
                                                    QSC / (WSC * MSC),
                                                    None, op0=Alu.mult)
                vN8 = ws.tile([128, NT, H], fp8, tag="vN8_ych")
                for t in range(NT):
                    vps = ps2.tile([128, CHUNK], f32, tag="big")
                    for n in range(2):
                        for i in range(KP):
                            nc.tensor.matmul(
                                vps[:, n * 512:(n + 1) * 512],
                                mixT8[:, 2 * i:2 * i + 2, t * 128:(t + 1) * 128],
                                wv_sb[:, 2 * i:2 * i + 2, n * 512:(n + 1) * 512],
                                start=(i == 0), stop=(i == KP - 1), perf_mode=DR)
                    nc.scalar.activation(vN8[:, t, :], vps[:],
                                         Act.Copy, scale=QSC / (WSC * MSC))

                # ---------- stage 5: attention ----------
                # scores arrive as 16*(q.k); exp folds hd^-0.5/16
                ocat = ws.tile([128, NT, H], fp16, tag="ocat_res")
                for h in range(NUM_HEADS):
                    et = ws.tile([128, KH, CHUNK], fp8, tag="xts_et", bufs=2)
                    for kt in range(NT):
                        stp = ps2.tile([128, CHUNK], f32, tag="big")
                        for qn in range(2):
                            nc.tensor.matmul(
                                stp[:, qn * 512:(qn + 1) * 512],
                                kT8[:, 2 * h:2 * h + 2, kt * 128:(kt + 1) * 128],
                                qT8[:, 2 * h:2 * h + 2, qn * 512:(qn + 1) * 512],
                                start=True, stop=True, perf_mode=DR)
                        nc.scalar.activation(et[:, kt, :], stp[:], Act.Exp,
                                             scale=float(HD ** -0.5) / (QSC * QSC))
                    for qt in range(NT):
                        ovp = ps.tile([128, 512], f32, tag="mm")
                        for i in range(KP):
                            nc.tensor.matmul(
                                ovp[:, :HD], et[:, 2 * i:2 * i + 2, qt * 128:(qt + 1) * 128],
                                vN8[:, 2 * i:2 * i + 2, h * HD:(h + 1) * HD],
                                start=(i == 0), stop=(i == KP - 1), perf_mode=DR)
                            # denominator column: same et weights, 0.25-vector.
                            # start=False even at i==0: the V-matmul's bank
                            # clear leaves has_written=0 here, so i==0
                            # overwrites rather than accumulates.
                            nc.tensor.matmul(
                                ovp[:, HD:HD + 1],
                                et[:, 2 * i:2 * i + 2, qt * 128:(qt + 1) * 128],
                                ones_sb[:],
                                start=False, stop=(i == KP - 1),
                                perf_mode=DR, skip_group_check=True)
                        rq = sm.tile([128, 1], f32, tag="rq")
                        nc.vector.reciprocal(rq[:], ovp[:, HD:HD + 1])
                        # ocat = (4*O) * 1/(s/4) = 16*o_norm
                        nc.vector.tensor_scalar(ocat[:, qt, h * HD:(h + 1) * HD],
                                                ovp[:, :HD], rq[:], None,
                                                op0=Alu.mult)
                otc16 = ws.tile([128, KH, CHUNK], fp16, tag="mixT_otc", bufs=2)
                for qt in range(NT):
                    nc.sync.dma_start_transpose(otc16[:, :, qt * 128:(qt + 1) * 128],
                                                ocat[:, qt, :])
                # convert per-qt column slice so wo(t=qt) starts as soon as
                # its transpose lands (pipelines the attention->wo boundary)
                otc8 = ws.tile([128, KH, CHUNK], fp8, tag="hidT_qT_otc8", bufs=2)
                for qt in range(NT):
                    eng = nc.gpsimd if qt % 2 == 0 else nc.vector
                    eng.tensor_copy(otc8[:, :, qt * 128:(qt + 1) * 128],
                                    otc16[:, :, qt * 128:(qt + 1) * 128])

                # ---------- stage 6: wo proj + residual + out LN ----------
                res = ws.tile([128, NT, H], fp16, tag="ocat_res")
                zT = ws.tile([128, KH, CHUNK], fp16, tag="kT_zT")
                mva6 = sm.tile([128, NT, 2], f32, tag="mva6")
                iva6 = sm.tile([128, NT], f32, tag="iva6")
                for t in range(NT):
                    ops_ = ps2.tile([128, CHUNK], f32, tag="big")
                    for n in range(2):
                        for i in range(KP):
                            nc.tensor.matmul(
                                ops_[:, n * 512:(n + 1) * 512],
                                otc8[:, 2 * i:2 * i + 2, t * 128:(t + 1) * 128],
                                wo_sb[:, 2 * i:2 * i + 2, n * 512:(n + 1) * 512],
                                start=(i == 0), stop=(i == KP - 1), perf_mode=DR)
                    # res = psum/(16*16) + mixN
                    nc.vector.scalar_tensor_tensor(
                        res[:, t, :], ops_[:], 1.0 / (OSC * WSC),
                        mixN[:, t, :], op0=Alu.mult, op1=Alu.add)
                    st6 = sm.tile([128, 2, 6], f32, tag="st6b")
                    for half in range(2):
                        nc.vector.bn_stats(st6[:, half, :],
                                           res[:, t, half * 512:(half + 1) * 512])
                    nc.vector.bn_aggr(mva6[:, t, :], st6[:])
                    if t % 2 == 1:
                        g = t // 2
                        u2 = sm.tile([128, 2], f32, tag="u2")
                        nc.scalar.activation(u2[:], mva6[:, g * 2:(g + 1) * 2, 1:2],
                                             Act.Sqrt, bias=eps_sb[:])
                        nc.vector.reciprocal(iva6[:, g * 2:(g + 1) * 2], u2[:])
                        for tt in (t - 1, t):
                            zt = sm.tile([128, CHUNK], fp16, tag="zt")
                            nc.vector.tensor_scalar(zt[:], res[:, tt, :],
                                                    mva6[:, tt, 0:1],
                                                    iva6[:, tt:tt + 1],
                                                    op0=Alu.subtract, op1=Alu.mult)
                            nc.sync.dma_start_transpose(
                                zT[:, :, tt * 128:(tt + 1) * 128], zt[:])

                # ---------- stage 7: output projection ----------
                ych = ws.tile([128, NT, G], f32, tag="vN8_ych")
                yps = None
                for t in range(NT):
                    if t % 2 == 0:
                        yps = ps.tile([128, 2, G], f32, tag="mm")
                    for fi in range(KH):
                        nc.tensor.matmul(yps[:, t % 2, :], zT[:, fi, t * 128:(t + 1) * 128],
                                         gw_sb[:, fi, :],
                                         start=(fi == 0), stop=(fi == KH - 1))
                    if t % 2 == 1:
                        nc.vector.tensor_add(ych[:, t - 1:t + 1, :], yps[:], bw2_sb[:])
                    if t % 4 == 3:
                        hh = t // 4
                        nc.sync.dma_start(
                            y.ap()[c, hh * 512:(hh + 1) * 512, :].rearrange(
                                "(t p) g -> p t g", p=128),
                            ych[:, hh * 4:(hh + 1) * 4, :])

    nc.compile()
    return nc


def _get_compiled():
    global _COMPILED
    if _COMPILED is None:
        _COMPILED = _build()
    return _COMPILED


def _prep_inputs(inputs):
    f32 = np.float32

    def a(name):
        return np.asarray(inputs[name], dtype=f32)

    x = a("x")
    mw = a("mother_wavelets")
    scales = a("scales")
    norm = np.sqrt(np.sum(mw ** 2, axis=2, keepdims=True))
    kern = (mw / np.maximum(norm, 1e-12)) * (1.0 / (1.0 + np.exp(-scales)))
    kern = kern[0, :, :, 0]                      # (W, H)
    kernT = np.ascontiguousarray(kern.T).astype(F16)

    w1a = np.concatenate([a("mix_w1"), a("mix_b1")[None, :]], axis=0).astype(F16)
    gln = np.ascontiguousarray(a("mix_ln_g").reshape(KM, 128).T).astype(f32)
    bln = np.ascontiguousarray(a("mix_ln_b").reshape(KM, 128).T).astype(f32)
    w2 = a("mix_w2").astype(F16)
    b2c = np.ascontiguousarray(a("mix_b2").reshape(KH, 128).T).astype(f32)
    gw = (a("out_ln_g")[:, None] * a("out_w")).astype(F16)
    bw_vec = a("out_ln_b") @ a("out_w") + a("out_b")
    bw = np.tile(bw_vec[None, :], (128, 1)).astype(f32)

    shared = {
        "kernt": kernT, "w1a": w1a, "gln": gln, "bln": bln, "w2": w2,
        "b2c": b2c,
        "wq8": (a("wq") * WSC).astype(FP8), "wk8": (a("wk") * WSC).astype(FP8),
        "wv8": (a("wv") * WSC).astype(FP8), "wo8": (a("wo") * WSC).astype(FP8),
        "gw": gw, "bw": bw,
    }

    xc = x.reshape(N_CHUNKS, CHUNK, H)
    xt_all = np.ascontiguousarray(xc.transpose(0, 2, 1)).astype(F16)  # (16, H, CHUNK)
    in_maps = []
    for core in range(N_CORES):
        m = dict(shared)
        m["xt"] = np.ascontiguousarray(xt_all[core * CPC:(core + 1) * CPC])
        in_maps.append(m)
    return in_maps


def kernel(**inputs) -> np.ndarray:
    from concourse.bass_utils import run_bass_kernel_spmd

    nc = _get_compiled()
    in_maps = _prep_inputs(inputs)
    res = run_bass_kernel_spmd(nc, in_maps, core_ids=list(range(N_CORES)))
    out = np.concatenate([r["y"] for r in res.results], axis=0)  # (16, CHUNK, G)
    return out.reshape(B, S, G).astype(np.float32)


# revision 61
# speedup vs baseline: 1.4765x; 1.0455x over previous
"""Trainium2 Bass kernel for nn_EntropyLM (wavelet-coeff mixer + chunked MHA + output proj).

Strategy: data-parallel over the 16 independent (batch x chunk) blocks, 2 per
NeuronCore.  The error-critical path (wavelet coeffs, mixer, output proj and
all 16-bit storage) runs in fp16 (same PE cost as bf16, 7.5x less rounding
error).  The attention path (q/k/v projections, scores, PV, wo) runs in
fp8e4m3 with DoubleRow matmuls: contraction pairs are packed into the PE's
double-pumped fp8 mode, which both halves instruction count and doubles
throughput.  Attention tolerates fp8 because the scores here are tiny
(std ~0.04), softmax is near-uniform, and the whole attention output is a
small additive correction to the residual stream.

All fp8 tensors are pre-scaled by exact powers of two (weights x16,
mixT x8, q/k/v x4, ocat x16) to lift values out of e4m3's subnormal range;
compensating 2^-k factors fold into existing evacuation instructions
(Act scale immediates / tensor_scalar scalars), so the rescaling is free.

Layout convention per chunk (CHUNK=1024 tokens, H=1024 features):
  * Linear layers contract over features -> feature-major operands
    ("T" tensors: [feat_part, token_free]); LN / softmax statistics run
    token-major.  DoubleRow pair slots hold consecutive 128-row contraction
    tiles: AP [128, 2, N] with pair stride = row stride of the tile.
  * Attention scores are computed directly transposed (ST = K @ Q^T,
    [k_part, q_free]); exp(ST) in fp8 is exactly the lhsT operand of the PV
    matmul.  The softmax denominator rides as a 1-column DoubleRow matmul
    against a constant 0.25 vector, reusing the et weights.
  * 16-bit orientation changes go through the DMA xbar transpose engine.
"""

import numpy as np
import ml_dtypes

B, S, H, G, W = 4, 4096, 1024, 256, 8
CHUNK = 1024
NUM_HEADS = 4
HD = H // NUM_HEADS          # 256 per-head dim
HM = H // 2                  # 512 mixer hidden
N_CHUNKS = B * (S // CHUNK)  # 16 independent chunks
N_CORES = 8
CPC = N_CHUNKS // N_CORES    # 2 chunks per core
NT = CHUNK // 128            # 8 token tiles
KH = H // 128                # 8 feature tiles (H)
KM = HM // 128               # 4 feature tiles (HM)
KP = KH // 2                 # 4 DoubleRow pair tiles (contraction H)
EPS = 1e-5
F16 = np.float16
FP8 = ml_dtypes.float8_e4m3  # IEEE e4m3 (max 240) == TRN fp8_e4m3

# power-of-2 fp8 pre-scales (all exact in fp)
WSC = 16.0                   # wq/wk/wv/wo uploaded as w*16
MSC = 8.0                    # mixT8 = mixT*8
QSC = 4.0                    # qT8/kT8/vN8 hold 4*q etc; evac scale 4/(16*8)
OSC = 16.0                   # ocat holds 16*o_norm (ones=0.25 arranges this)

_COMPILED = None


def _build(debug=False):
    import concourse.bass as bass  # noqa: F401
    import concourse.tile as tile
    from concourse import bacc, mybir

    fp16 = mybir.dt.float16
    fp8 = mybir.dt.float8e4
    f32 = mybir.dt.float32
    Alu = mybir.AluOpType
    Act = mybir.ActivationFunctionType
    DR = mybir.MatmulPerfMode.DoubleRow

    nc = bacc.Bacc("TRN2", target_bir_lowering=False, debug=False,
                   enable_asserts=True, num_devices=N_CORES)

    # ---- DRAM tensors (per-core views; same NEFF on all 8 cores) ----
    xt = nc.dram_tensor("xt", [CPC, H, CHUNK], fp16, kind="ExternalInput")
    kernT = nc.dram_tensor("kernt", [H, W], fp16, kind="ExternalInput")
    w1a = nc.dram_tensor("w1a", [W + 1, HM], fp16, kind="ExternalInput")
    gln = nc.dram_tensor("gln", [128, KM], f32, kind="ExternalInput")
    bln = nc.dram_tensor("bln", [128, KM], f32, kind="ExternalInput")
    w2 = nc.dram_tensor("w2", [HM, H], fp16, kind="ExternalInput")
    b2c = nc.dram_tensor("b2c", [128, KH], f32, kind="ExternalInput")
    wq8 = nc.dram_tensor("wq8", [H, H], fp8, kind="ExternalInput")
    wk8 = nc.dram_tensor("wk8", [H, H], fp8, kind="ExternalInput")
    wv8 = nc.dram_tensor("wv8", [H, H], fp8, kind="ExternalInput")
    wo8 = nc.dram_tensor("wo8", [H, H], fp8, kind="ExternalInput")
    gw = nc.dram_tensor("gw", [H, G], fp16, kind="ExternalInput")
    bw = nc.dram_tensor("bw", [128, G], f32, kind="ExternalInput")
    y = nc.dram_tensor("y", [CPC, CHUNK, G], f32, kind="ExternalOutput")

    with tile.TileContext(nc) as tc:
        with (
            tc.tile_pool(name="wp", bufs=1) as wp,
            tc.tile_pool(name="ws", bufs=1) as ws,
            tc.tile_pool(name="sm", bufs=2) as sm,
            tc.tile_pool(name="ps", bufs=4, space="PSUM") as ps,
            tc.tile_pool(name="ps2", bufs=2, space="PSUM") as ps2,
        ):
            # ---------- input x first (so coeffs can start before the big
            # weight DMAs drain), then persistent weights ----------
            kt_sb = wp.tile([128, KH, W], fp16, tag="ktw")
            nc.sync.dma_start(kt_sb[:], kernT.ap().rearrange("(i p) w -> p i w", p=128))
            xts_all = []
            for c in range(CPC):
                xts = ws.tile([128, KH, CHUNK], fp16, tag="xts_et", bufs=2)
                for ii in range(2):
                    nc.sync.dma_start(
                        xts[:, ii * 4:(ii + 1) * 4, :],
                        xt.ap()[c, ii * 512:(ii + 1) * 512, :].rearrange(
                            "(i p) t -> p i t", p=128))
                xts_all.append(xts)
            w1a_sb = wp.tile([W + 1, HM], fp16, tag="w1a")
            nc.sync.dma_start(w1a_sb[:], w1a.ap())
            gln_sb = wp.tile([128, KM], f32, tag="gln")
            nc.sync.dma_start(gln_sb[:], gln.ap())
            bln_sb = wp.tile([128, KM], f32, tag="bln")
            nc.sync.dma_start(bln_sb[:], bln.ap())
            b2_sb = wp.tile([128, KH], f32, tag="b2")
            nc.sync.dma_start(b2_sb[:], b2c.ap())
            gw_sb = wp.tile([128, KH, G], fp16, tag="gw")
            nc.sync.dma_start(gw_sb[:], gw.ap().rearrange("(i p) g -> p i g", p=128))
            bw2_sb = wp.tile([128, 2, G], f32, tag="bw")
            nc.sync.dma_start(bw2_sb[:, 0, :], bw.ap())
            nc.sync.dma_start(bw2_sb[:, 1, :], bw.ap())
            ones_sb = wp.tile([128, 2, 1], fp8, tag="ones")
            nc.vector.memset(ones_sb[:], 0.25)
            eps_sb = wp.tile([128, 1], f32, tag="eps")
            nc.vector.memset(eps_sb[:], EPS)
            # fp8 weights resident across both chunks
            wq_sb = wp.tile([128, KH, H], fp8, tag="wq8")
            nc.sync.dma_start(wq_sb[:], wq8.ap().rearrange("(i p) m -> p i m", p=128))
            wk_sb = wp.tile([128, KH, H], fp8, tag="wk8")
            nc.sync.dma_start(wk_sb[:], wk8.ap().rearrange("(i p) m -> p i m", p=128))
            wv_sb = wp.tile([128, KH, H], fp8, tag="wv8")
            nc.sync.dma_start(wv_sb[:], wv8.ap().rearrange("(i p) m -> p i m", p=128))
            wo_sb = wp.tile([128, KH, H], fp8, tag="wo8")
            nc.sync.dma_start(wo_sb[:], wo8.ap().rearrange("(i p) m -> p i m", p=128))
            w2_sb = wp.tile([128, KM, H], fp16, tag="w2")
            nc.sync.dma_start(w2_sb[:], w2.ap().rearrange("(i p) m -> p i m", p=128))

            # ---------- stage 1 (both chunks up front): wavelet coeffs ----------
            coefs = []
            for c in range(CPC):
                xts = xts_all[c]
                coef = ws.tile([W + 1, CHUNK], fp16, tag="coef", bufs=2)
                # row W is the constant 1.0 bias row for the folded mix_b1
                nc.gpsimd.memset(coef[:, :], 1.0)
                for n in range(2):
                    cps = ps.tile([128, 512], f32, tag="mm")
                    for i in range(KH):
                        nc.tensor.matmul(cps[:W, :], kt_sb[:, i, :],
                                         xts[:, i, n * 512:(n + 1) * 512],
                                         start=(i == 0), stop=(i == KH - 1))
                    nc.vector.tensor_copy(coef[:W, n * 512:(n + 1) * 512], cps[:W, :])
                coefs.append(coef)

            for c in range(CPC):
                coef = coefs[c]
                # ---------- stage 2: mixer hidden + LN + gelu -> hidT ----------
                # sqrt is batched per 4 tiles (one table-friendly Act op);
                # the normalize itself runs on Act as Identity(iv*x - m*iv).
                hidT = ws.tile([128, KM, CHUNK], fp16, tag="hidT_qT_otc8", bufs=2)
                hpss = {}
                mva = sm.tile([128, NT, 2], f32, tag="mva")
                iva = sm.tile([128, NT], f32, tag="iva")
                for t in range(NT):
                    hps = ps.tile([128, 512], f32, tag="mm")
                    nc.tensor.matmul(hps[:], coef[:, t * 128:(t + 1) * 128],
                                     w1a_sb[:], start=True, stop=True)
                    hpss[t] = hps
                    st6 = sm.tile([128, 6], f32, tag="st6")
                    nc.vector.bn_stats(st6[:], hps[:])
                    nc.vector.bn_aggr(mva[:, t, :], st6[:])
                    if t % 2 == 1:
                        g = t // 2
                        u2 = sm.tile([128, 2], f32, tag="u2")
                        nc.scalar.activation(u2[:], mva[:, g * 2:(g + 1) * 2, 1:2],
                                             Act.Sqrt, bias=eps_sb[:])
                        nc.vector.reciprocal(iva[:, g * 2:(g + 1) * 2], u2[:])
                        for tt in (t - 1, t):
                            tmp = sm.tile([128, HM], fp16, tag="mtmp")
                            nc.vector.tensor_scalar(tmp[:], hpss.pop(tt)[:],
                                                    mva[:, tt, 0:1],
                                                    iva[:, tt:tt + 1],
                                                    op0=Alu.subtract, op1=Alu.mult)
                            nc.sync.dma_start_transpose(
                                hidT[:, :, tt * 128:(tt + 1) * 128], tmp[:])
                # gamma/beta (per-partition feature-major) fused into gelu:
                # gelu(g*x + b) in one Act pass
                for nh in range(2):
                    for ki in range(KM):
                        sl = hidT[:, ki, nh * 512:(nh + 1) * 512]
                        nc.scalar.activation(sl, sl, Act.Gelu,
                                             scale=gln_sb[:, ki:ki + 1],
                                             bias=bln_sb[:, ki:ki + 1])

                # ---------- stage 3: mixedT (+b2), mixT8, mixed_nat ----------
                mixT = ws.tile([128, KH, CHUNK], fp16, tag="mixT_otc", bufs=2)
                mixT8 = ws.tile([128, KH, CHUNK], fp8, tag="mix8")
                # n-major so the n=0 half's matmuls can start as soon as the
                # first half of stage 2's transposes land (fills the S2 bubble)
                for n in range(2):
                    for m in range(KH):
                        mps = ps.tile([128, 512], f32, tag="mm")
                        for ki in range(KM):
                            nc.tensor.matmul(mps[:],
                                             w2_sb[:, ki, m * 128:(m + 1) * 128],
                                             hidT[:, ki, n * 512:(n + 1) * 512],
                                             start=(ki == 0), stop=(ki == KM - 1))
                        sl = mixT[:, m, n * 512:(n + 1) * 512]
                        nc.scalar.activation(sl, mps[:], Act.Identity,
                                             bias=b2_sb[:, m:m + 1])
                        nc.gpsimd.tensor_scalar(mixT8[:, m, n * 512:(n + 1) * 512],
                                                sl, MSC, None, op0=Alu.mult)
                mixN = ws.tile([128, NT, H], fp16, tag="mixN")
                for m in range(KH):
                    nc.sync.dma_start_transpose(mixN[:, :, m * 128:(m + 1) * 128],
                                                mixT[:, m, :])

                # ---------- stage 4: qT8, kT8 (feature-major), vN8 ----------
                qT8 = ws.tile([128, KH, CHUNK], fp8, tag="hidT_qT_otc8", bufs=2)
                kT8 = ws.tile([128, KH, CHUNK], fp8, tag="kT_zT")
                for (dst, wsb, on_act) in ((qT8, wq_sb, True), (kT8, wk_sb, False)):
                    for m in range(KH):
                        for n in range(2):
                            qps = ps.tile([128, 512], f32, tag="mm")
                            for i in range(KP):
                                nc.tensor.matmul(
                                    qps[:],
                                    wsb[:, 2 * i:2 * i + 2, m * 128:(m + 1) * 128],
                                    mixT8[:, 2 * i:2 * i + 2, n * 512:(n + 1) * 512],
                                    start=(i == 0), stop=(i == KP - 1), perf_mode=DR)
                            sl = dst[:, m, n * 512:(n + 1) * 512]
                            if on_act:
                                nc.scalar.activation(sl, qps[:], Act.Copy,
                                                     scale=QSC / (WSC * MSC))
                            else:
                                nc.vector.tensor_scalar(sl, qps[:],
                                                        QSC / (WSC * MSC),
                                                        None, op0=Alu.mult)
                vN8 = ws.tile([128, NT, H], fp8, tag="vN8_ych")
                for t in range(NT):
                    for n in range(2):
                        vps = ps.tile([128, 512], f32, tag="mm")
                        for i in range(KP):
                            nc.tensor.matmul(
                                vps[:],
                                mixT8[:, 2 * i:2 * i + 2, t * 128:(t + 1) * 128],
                                wv_sb[:, 2 * i:2 * i + 2, n * 512:(n + 1) * 512],
                                start=(i == 0), stop=(i == KP - 1), perf_mode=DR)
                        nc.scalar.activation(vN8[:, t, n * 512:(n + 1) * 512],
                                             vps[:], Act.Copy,
                                             scale=QSC / (WSC * MSC))

                # ---------- stage 5: attention ----------
                # scores arrive as 16*(q.k); exp folds hd^-0.5/16
                ocat = ws.tile([128, NT, H], fp16, tag="ocat_res")
                for h in range(NUM_HEADS):
                    et = ws.tile([128, KH, CHUNK], fp8, tag="xts_et", bufs=2)
                    for kt in range(NT):
                        stp = ps2.tile([128, CHUNK], f32, tag="big")
                        for qn in range(2):
                            nc.tensor.matmul(
                                stp[:, qn * 512:(qn + 1) * 512],
                                kT8[:, 2 * h:2 * h + 2, kt * 128:(kt + 1) * 128],
                                qT8[:, 2 * h:2 * h + 2, qn * 512:(qn + 1) * 512],
                                start=True, stop=True, perf_mode=DR)
                        nc.scalar.activation(et[:, kt, :], stp[:], Act.Exp,
                                             scale=float(HD ** -0.5) / (QSC * QSC))
                    for qt in range(NT):
                        ovp = ps.tile([128, 512], f32, tag="mm")
                        for i in range(KP):
                            nc.tensor.matmul(
                                ovp[:, :HD], et[:, 2 * i:2 * i + 2, qt * 128:(qt + 1) * 128],
                                vN8[:, 2 * i:2 * i + 2, h * HD:(h + 1) * HD],
                                start=(i == 0), stop=(i == KP - 1), perf_mode=DR)
                            # denominator column: same et weights, 0.25-vector.
                            # start=False even at i==0: the V-matmul's bank
                            # clear leaves has_written=0 here, so i==0
                            # overwrites rather than accumulates.
                            nc.tensor.matmul(
                                ovp[:, HD:HD + 1],
                                et[:, 2 * i:2 * i + 2, qt * 128:(qt + 1) * 128],
                                ones_sb[:],
                                start=False, stop=(i == KP - 1),
                                perf_mode=DR, skip_group_check=True)
                        rq = sm.tile([128, 1], f32, tag="rq")
                        nc.vector.reciprocal(rq[:], ovp[:, HD:HD + 1])
                        # ocat = (4*O) * 1/(s/4) = 16*o_norm
                        nc.vector.tensor_scalar(ocat[:, qt, h * HD:(h + 1) * HD],
                                                ovp[:, :HD], rq[:], None,
                                                op0=Alu.mult)
                otc16 = ws.tile([128, KH, CHUNK], fp16, tag="mixT_otc", bufs=2)
                for qt in range(NT):
                    nc.sync.dma_start_transpose(otc16[:, :, qt * 128:(qt + 1) * 128],
                                                ocat[:, qt, :])
                # convert per-qt column slice so wo(t=qt) starts as soon as
                # its transpose lands (pipelines the attention->wo boundary)
                otc8 = ws.tile([128, KH, CHUNK], fp8, tag="hidT_qT_otc8", bufs=2)
                for qt in range(NT):
                    eng = nc.gpsimd if qt % 2 == 0 else nc.vector
                    eng.tensor_copy(otc8[:, :, qt * 128:(qt + 1) * 128],
                                    otc16[:, :, qt * 128:(qt + 1) * 128])

                # ---------- stage 6: wo proj + residual + out LN ----------
                res = ws.tile([128, NT, H], fp16, tag="ocat_res")
                zT = ws.tile([128, KH, CHUNK], fp16, tag="kT_zT")
                mva6 = sm.tile([128, NT, 2], f32, tag="mva6")
                iva6 = sm.tile([128, NT], f32, tag="iva6")
                for t in range(NT):
                    for n in range(2):
                        ops_ = ps.tile([128, 512], f32, tag="mm")
                        for i in range(KP):
                            nc.tensor.matmul(
                                ops_[:],
                                otc8[:, 2 * i:2 * i + 2, t * 128:(t + 1) * 128],
                                wo_sb[:, 2 * i:2 * i + 2, n * 512:(n + 1) * 512],
                                start=(i == 0), stop=(i == KP - 1), perf_mode=DR)
                        # res = psum/(16*16) + mixN
                        nc.vector.scalar_tensor_tensor(
                            res[:, t, n * 512:(n + 1) * 512], ops_[:],
                            1.0 / (OSC * WSC),
                            mixN[:, t, n * 512:(n + 1) * 512],
                            op0=Alu.mult, op1=Alu.add)
                    st6 = sm.tile([128, 2, 6], f32, tag="st6b")
                    for half in range(2):
                        nc.vector.bn_stats(st6[:, half, :],
                                           res[:, t, half * 512:(half + 1) * 512])
                    nc.vector.bn_aggr(mva6[:, t, :], st6[:])
                    if t % 2 == 1:
                        g = t // 2
                        u2 = sm.tile([128, 2], f32, tag="u2")
                        nc.scalar.activation(u2[:], mva6[:, g * 2:(g + 1) * 2, 1:2],
                                             Act.Sqrt, bias=eps_sb[:])
                        nc.vector.reciprocal(iva6[:, g * 2:(g + 1) * 2], u2[:])
                        for tt in (t - 1, t):
                            zt = sm.tile([128, CHUNK], fp16, tag="zt")
                            nc.vector.tensor_scalar(zt[:], res[:, tt, :],
                                                    mva6[:, tt, 0:1],
                                                    iva6[:, tt:tt + 1],
                                                    op0=Alu.subtract, op1=Alu.mult)
                            nc.sync.dma_start_transpose(
                                zT[:, :, tt * 128:(tt + 1) * 128], zt[:])

                # ---------- stage 7: output projection ----------
                ych = ws.tile([128, NT, G], f32, tag="vN8_ych")
                yps = None
                for t in range(NT):
                    if t % 2 == 0:
                        yps = ps.tile([128, 2, G], f32, tag="mm")
                    for fi in range(KH):
                        nc.tensor.matmul(yps[:, t % 2, :], zT[:, fi, t * 128:(t + 1) * 128],
                                         gw_sb[:, fi, :],
                                         start=(fi == 0), stop=(fi == KH - 1))
                    if t % 2 == 1:
                        nc.vector.tensor_add(ych[:, t - 1:t + 1, :], yps[:], bw2_sb[:])
                    if t % 4 == 3:
                        hh = t // 4
                        nc.sync.dma_start(
                            y.ap()[c, hh * 512:(hh + 1) * 512, :].rearrange(
                                "(t p) g -> p t g", p=128),
                            ych[:, hh * 4:(hh + 1) * 4, :])

    nc.compile()
    return nc


def _get_compiled():
    global _COMPILED
    if _COMPILED is None:
        _COMPILED = _build()
    return _COMPILED


def _prep_inputs(inputs):
    f32 = np.float32

    def a(name):
        return np.asarray(inputs[name], dtype=f32)

    x = a("x")
    mw = a("mother_wavelets")
    scales = a("scales")
    norm = np.sqrt(np.sum(mw ** 2, axis=2, keepdims=True))
    kern = (mw / np.maximum(norm, 1e-12)) * (1.0 / (1.0 + np.exp(-scales)))
    kern = kern[0, :, :, 0]                      # (W, H)
    kernT = np.ascontiguousarray(kern.T).astype(F16)

    w1a = np.concatenate([a("mix_w1"), a("mix_b1")[None, :]], axis=0).astype(F16)
    gln = np.ascontiguousarray(a("mix_ln_g").reshape(KM, 128).T).astype(f32)
    bln = np.ascontiguousarray(a("mix_ln_b").reshape(KM, 128).T).astype(f32)
    w2 = a("mix_w2").astype(F16)
    b2c = np.ascontiguousarray(a("mix_b2").reshape(KH, 128).T).astype(f32)
    gw = (a("out_ln_g")[:, None] * a("out_w")).astype(F16)
    bw_vec = a("out_ln_b") @ a("out_w") + a("out_b")
    bw = np.tile(bw_vec[None, :], (128, 1)).astype(f32)

    shared = {
        "kernt": kernT, "w1a": w1a, "gln": gln, "bln": bln, "w2": w2,
        "b2c": b2c,
        "wq8": (a("wq") * WSC).astype(FP8), "wk8": (a("wk") * WSC).astype(FP8),
        "wv8": (a("wv") * WSC).astype(FP8), "wo8": (a("wo") * WSC).astype(FP8),
        "gw": gw, "bw": bw,
    }

    xc = x.reshape(N_CHUNKS, CHUNK, H)
    xt_all = np.ascontiguousarray(xc.transpose(0, 2, 1)).astype(F16)  # (16, H, CHUNK)
    in_maps = []
    for core in range(N_CORES):
        m = dict(shared)
        m["xt"] = np.ascontiguousarray(xt_all[core * CPC:(core + 1) * CPC])
        in_maps.append(m)
    return in_maps


def kernel(**inputs) -> np.ndarray:
    from concourse.bass_utils import run_bass_kernel_spmd

    nc = _get_compiled()
    in_maps = _prep_inputs(inputs)
    res = run_bass_kernel_spmd(nc, in_maps, core_ids=list(range(N_CORES)))
    out = np.concatenate([r["y"] for r in res.results], axis=0)  # (16, CHUNK, G)
    return out.reshape(B, S, G).astype(np.float32)


# revision 81
# speedup vs baseline: 1.4798x; 1.0022x over previous
"""Trainium2 Bass kernel for nn_EntropyLM (wavelet-coeff mixer + chunked MHA + output proj).

Strategy: data-parallel over the 16 independent (batch x chunk) blocks, 2 per
NeuronCore.  The error-critical path (wavelet coeffs, mixer, output proj and
all 16-bit storage) runs in fp16 (same PE cost as bf16, 7.5x less rounding
error).  The attention path (q/k/v projections, scores, PV, wo) runs in
fp8e4m3 with DoubleRow matmuls: contraction pairs are packed into the PE's
double-pumped fp8 mode, which both halves instruction count and doubles
throughput.  Attention tolerates fp8 because the scores here are tiny
(std ~0.04), softmax is near-uniform, and the whole attention output is a
small additive correction to the residual stream.

All fp8 tensors are pre-scaled by exact powers of two (weights x16,
mixT x8, q/k/v x4, ocat x16) to lift values out of e4m3's subnormal range;
compensating 2^-k factors fold into existing evacuation instructions
(Act scale immediates / tensor_scalar scalars), so the rescaling is free.

Layout convention per chunk (CHUNK=1024 tokens, H=1024 features):
  * Linear layers contract over features -> feature-major operands
    ("T" tensors: [feat_part, token_free]); LN / softmax statistics run
    token-major.  DoubleRow pair slots hold consecutive 128-row contraction
    tiles: AP [128, 2, N] with pair stride = row stride of the tile.
  * Attention scores are computed directly transposed (ST = K @ Q^T,
    [k_part, q_free]); exp(ST) in fp8 is exactly the lhsT operand of the PV
    matmul.  The softmax denominator rides as a 1-column DoubleRow matmul
    against a constant 0.25 vector, reusing the et weights.
  * 16-bit orientation changes go through the DMA xbar transpose engine.
"""

import numpy as np
import ml_dtypes

B, S, H, G, W = 4, 4096, 1024, 256, 8
CHUNK = 1024
NUM_HEADS = 4
HD = H // NUM_HEADS          # 256 per-head dim
HM = H // 2                  # 512 mixer hidden
N_CHUNKS = B * (S // CHUNK)  # 16 independent chunks
N_CORES = 8
CPC = N_CHUNKS // N_CORES    # 2 chunks per core
NT = CHUNK // 128            # 8 token tiles
KH = H // 128                # 8 feature tiles (H)
KM = HM // 128               # 4 feature tiles (HM)
KP = KH // 2                 # 4 DoubleRow pair tiles (contraction H)
EPS = 1e-5
F16 = np.float16
FP8 = ml_dtypes.float8_e4m3  # IEEE e4m3 (max 240) == TRN fp8_e4m3

# power-of-2 fp8 pre-scales (all exact in fp)
WSC = 16.0                   # wq/wk/wv/wo uploaded as w*16
MSC = 8.0                    # mixT8 = mixT*8
QSC = 4.0                    # qT8/kT8/vN8 hold 4*q etc; evac scale 4/(16*8)
OSC = 16.0                   # ocat holds 16*o_norm (ones=0.25 arranges this)

_COMPILED = None


def _build(debug=False):
    import concourse.bass as bass  # noqa: F401
    import concourse.tile as tile
    from concourse import bacc, mybir

    fp16 = mybir.dt.float16
    fp8 = mybir.dt.float8e4
    f32 = mybir.dt.float32
    Alu = mybir.AluOpType
    Act = mybir.ActivationFunctionType
    DR = mybir.MatmulPerfMode.DoubleRow

    nc = bacc.Bacc("TRN2", target_bir_lowering=False, debug=False,
                   enable_asserts=True, num_devices=N_CORES)

    # ---- DRAM tensors (per-core views; same NEFF on all 8 cores) ----
    xt = nc.dram_tensor("xt", [CPC, H, CHUNK], fp16, kind="ExternalInput")
    kernT = nc.dram_tensor("kernt", [H, W], fp16, kind="ExternalInput")
    w1a = nc.dram_tensor("w1a", [W + 1, HM], fp16, kind="ExternalInput")
    gln = nc.dram_tensor("gln", [128, KM], f32, kind="ExternalInput")
    bln = nc.dram_tensor("bln", [128, KM], f32, kind="ExternalInput")
    w2 = nc.dram_tensor("w2", [HM, H], fp16, kind="ExternalInput")
    b2c = nc.dram_tensor("b2c", [128, KH], f32, kind="ExternalInput")
    wq8 = nc.dram_tensor("wq8", [H, H], fp8, kind="ExternalInput")
    wk8 = nc.dram_tensor("wk8", [H, H], fp8, kind="ExternalInput")
    wv8 = nc.dram_tensor("wv8", [H, H], fp8, kind="ExternalInput")
    wo8 = nc.dram_tensor("wo8", [H, H], fp8, kind="ExternalInput")
    gw = nc.dram_tensor("gw", [H, G], fp16, kind="ExternalInput")
    bw = nc.dram_tensor("bw", [128, G], f32, kind="ExternalInput")
    y = nc.dram_tensor("y", [CPC, CHUNK, G], f32, kind="ExternalOutput")

    with tile.TileContext(nc) as tc:
        with (
            tc.tile_pool(name="wp", bufs=1) as wp,
            tc.tile_pool(name="ws", bufs=1) as ws,
            tc.tile_pool(name="sm", bufs=2) as sm,
            tc.tile_pool(name="ps", bufs=4, space="PSUM") as ps,
            tc.tile_pool(name="ps2", bufs=2, space="PSUM") as ps2,
        ):
            # ---------- input x first (so coeffs can start before the big
            # weight DMAs drain), then persistent weights ----------
            kt_sb = wp.tile([128, KH, W], fp16, tag="ktw")
            nc.sync.dma_start(kt_sb[:], kernT.ap().rearrange("(i p) w -> p i w", p=128))
            xts_all = []
            for c in range(CPC):
                xts = ws.tile([128, KH, CHUNK], fp16, tag="xts_et", bufs=2)
                for ii in range(2):
                    nc.sync.dma_start(
                        xts[:, ii * 4:(ii + 1) * 4, :],
                        xt.ap()[c, ii * 512:(ii + 1) * 512, :].rearrange(
                            "(i p) t -> p i t", p=128))
                xts_all.append(xts)
            w1a_sb = wp.tile([W + 1, HM], fp16, tag="w1a")
            nc.sync.dma_start(w1a_sb[:], w1a.ap())
            gln_sb = wp.tile([128, KM], f32, tag="gln")
            nc.sync.dma_start(gln_sb[:], gln.ap())
            bln_sb = wp.tile([128, KM], f32, tag="bln")
            nc.sync.dma_start(bln_sb[:], bln.ap())
            b2_sb = wp.tile([128, KH], f32, tag="b2")
            nc.sync.dma_start(b2_sb[:], b2c.ap())
            gw_sb = wp.tile([128, KH, G], fp16, tag="gw")
            nc.sync.dma_start(gw_sb[:], gw.ap().rearrange("(i p) g -> p i g", p=128))
            bw2_sb = wp.tile([128, 2, G], f32, tag="bw")
            nc.sync.dma_start(bw2_sb[:, 0, :], bw.ap())
            nc.sync.dma_start(bw2_sb[:, 1, :], bw.ap())
            ones_sb = wp.tile([128, 2, 1], fp8, tag="ones")
            nc.vector.memset(ones_sb[:], 0.25)
            eps_sb = wp.tile([128, 1], f32, tag="eps")
            nc.vector.memset(eps_sb[:], EPS)
            # fp8 weights resident across both chunks
            wq_sb = wp.tile([128, KH, H], fp8, tag="wq8")
            nc.sync.dma_start(wq_sb[:], wq8.ap().rearrange("(i p) m -> p i m", p=128))
            wk_sb = wp.tile([128, KH, H], fp8, tag="wk8")
            nc.sync.dma_start(wk_sb[:], wk8.ap().rearrange("(i p) m -> p i m", p=128))
            wv_sb = wp.tile([128, KH, H], fp8, tag="wv8")
            nc.sync.dma_start(wv_sb[:], wv8.ap().rearrange("(i p) m -> p i m", p=128))
            wo_sb = wp.tile([128, KH, H], fp8, tag="wo8")
            nc.sync.dma_start(wo_sb[:], wo8.ap().rearrange("(i p) m -> p i m", p=128))
            w2_sb = wp.tile([128, KM, H], fp16, tag="w2")
            nc.sync.dma_start(w2_sb[:], w2.ap().rearrange("(i p) m -> p i m", p=128))

            # ---------- stage 1 (both chunks up front): wavelet coeffs ----------
            coefs = []
            for c in range(CPC):
                xts = xts_all[c]
                coef = ws.tile([W + 1, CHUNK], fp16, tag="coef", bufs=2)
                # row W is the constant 1.0 bias row for the folded mix_b1
                nc.vector.memset(coef[:, :], 1.0)
                for n in range(2):
                    cps = ps.tile([128, 512], f32, tag="mm")
                    for i in range(KH):
                        nc.tensor.matmul(cps[:W, :], kt_sb[:, i, :],
                                         xts[:, i, n * 512:(n + 1) * 512],
                                         start=(i == 0), stop=(i == KH - 1))
                    nc.vector.tensor_copy(coef[:W, n * 512:(n + 1) * 512], cps[:W, :])
                coefs.append(coef)

            for c in range(CPC):
                coef = coefs[c]
                # ---------- stage 2: mixer hidden + LN + gelu -> hidT ----------
                # sqrt is batched per 4 tiles (one table-friendly Act op);
                # the normalize itself runs on Act as Identity(iv*x - m*iv).
                hidT = ws.tile([128, KM, CHUNK], fp16, tag="hidT_qT_otc8", bufs=2)
                hpss = {}
                mva = sm.tile([128, NT, 2], f32, tag="mva")
                iva = sm.tile([128, NT], f32, tag="iva")
                for t in range(NT):
                    hps = ps.tile([128, 512], f32, tag="mm")
                    nc.tensor.matmul(hps[:], coef[:, t * 128:(t + 1) * 128],
                                     w1a_sb[:], start=True, stop=True)
                    hpss[t] = hps
                    st6 = sm.tile([128, 6], f32, tag="st6")
                    nc.vector.bn_stats(st6[:], hps[:])
                    nc.vector.bn_aggr(mva[:, t, :], st6[:])
                    if t % 2 == 1:
                        g = t // 2
                        u2 = sm.tile([128, 2], f32, tag="u2")
                        nc.scalar.activation(u2[:], mva[:, g * 2:(g + 1) * 2, 1:2],
                                             Act.Sqrt, bias=eps_sb[:])
                        nc.vector.reciprocal(iva[:, g * 2:(g + 1) * 2], u2[:])
                        for tt in (t - 1, t):
                            tmp = sm.tile([128, HM], fp16, tag="mtmp")
                            nc.vector.tensor_scalar(tmp[:], hpss.pop(tt)[:],
                                                    mva[:, tt, 0:1],
                                                    iva[:, tt:tt + 1],
                                                    op0=Alu.subtract, op1=Alu.mult)
                            nc.sync.dma_start_transpose(
                                hidT[:, :, tt * 128:(tt + 1) * 128], tmp[:])
                # gamma/beta (per-partition feature-major) fused into gelu:
                # gelu(g*x + b) in one Act pass
                for nh in range(2):
                    for ki in range(KM):
                        sl = hidT[:, ki, nh * 512:(nh + 1) * 512]
                        nc.scalar.activation(sl, sl, Act.Gelu,
                                             scale=gln_sb[:, ki:ki + 1],
                                             bias=bln_sb[:, ki:ki + 1])

                # ---------- stage 3: mixedT (+b2), mixT8, mixed_nat ----------
                mixT = ws.tile([128, KH, CHUNK], fp16, tag="mixT_otc", bufs=2)
                mixT8 = ws.tile([128, KH, CHUNK], fp8, tag="mix8")
                # n-major so the n=0 half's matmuls can start as soon as the
                # first half of stage 2's transposes land (fills the S2 bubble)
                for n in range(2):
                    for m in range(KH):
                        mps = ps.tile([128, 512], f32, tag="mm")
                        for ki in range(KM):
                            nc.tensor.matmul(mps[:],
                                             w2_sb[:, ki, m * 128:(m + 1) * 128],
                                             hidT[:, ki, n * 512:(n + 1) * 512],
                                             start=(ki == 0), stop=(ki == KM - 1))
                        sl = mixT[:, m, n * 512:(n + 1) * 512]
                        nc.scalar.activation(sl, mps[:], Act.Identity,
                                             bias=b2_sb[:, m:m + 1])
                        nc.gpsimd.tensor_scalar(mixT8[:, m, n * 512:(n + 1) * 512],
                                                sl, MSC, None, op0=Alu.mult)
                mixN = ws.tile([128, NT, H], fp16, tag="mixN")
                for m in range(KH):
                    nc.sync.dma_start_transpose(mixN[:, :, m * 128:(m + 1) * 128],
                                                mixT[:, m, :])

                # ---------- stage 4: qT8, kT8 (feature-major), vN8 ----------
                qT8 = ws.tile([128, KH, CHUNK], fp8, tag="hidT_qT_otc8", bufs=2)
                kT8 = ws.tile([128, KH, CHUNK], fp8, tag="kT_zT")
                for (dst, wsb, on_act) in ((qT8, wq_sb, True), (kT8, wk_sb, False)):
                    for m in range(KH):
                        for n in range(2):
                            qps = ps.tile([128, 512], f32, tag="mm")
                            for i in range(KP):
                                nc.tensor.matmul(
                                    qps[:],
                                    wsb[:, 2 * i:2 * i + 2, m * 128:(m + 1) * 128],
                                    mixT8[:, 2 * i:2 * i + 2, n * 512:(n + 1) * 512],
                                    start=(i == 0), stop=(i == KP - 1), perf_mode=DR)
                            sl = dst[:, m, n * 512:(n + 1) * 512]
                            if on_act:
                                nc.scalar.activation(sl, qps[:], Act.Copy,
                                                     scale=QSC / (WSC * MSC))
                            else:
                                nc.vector.tensor_scalar(sl, qps[:],
                                                        QSC / (WSC * MSC),
                                                        None, op0=Alu.mult)
                vN8 = ws.tile([128, NT, H], fp8, tag="vN8_ych")
                for t in range(NT):
                    for n in range(2):
                        vps = ps.tile([128, 512], f32, tag="mm")
                        for i in range(KP):
                            nc.tensor.matmul(
                                vps[:],
                                mixT8[:, 2 * i:2 * i + 2, t * 128:(t + 1) * 128],
                                wv_sb[:, 2 * i:2 * i + 2, n * 512:(n + 1) * 512],
                                start=(i == 0), stop=(i == KP - 1), perf_mode=DR)
                        nc.scalar.activation(vN8[:, t, n * 512:(n + 1) * 512],
                                             vps[:], Act.Copy,
                                             scale=QSC / (WSC * MSC))

                # ---------- stage 5: attention ----------
                # scores arrive as 16*(q.k); exp folds hd^-0.5/16
                ocat = ws.tile([128, NT, H], fp16, tag="ocat_res")
                for h in range(NUM_HEADS):
                    et = ws.tile([128, KH, CHUNK], fp8, tag="xts_et", bufs=2)
                    for kt in range(NT):
                        stp = ps2.tile([128, CHUNK], f32, tag="big")
                        for qn in range(2):
                            nc.tensor.matmul(
                                stp[:, qn * 512:(qn + 1) * 512],
                                kT8[:, 2 * h:2 * h + 2, kt * 128:(kt + 1) * 128],
                                qT8[:, 2 * h:2 * h + 2, qn * 512:(qn + 1) * 512],
                                start=True, stop=True, perf_mode=DR)
                        nc.scalar.activation(et[:, kt, :], stp[:], Act.Exp,
                                             scale=float(HD ** -0.5) / (QSC * QSC))
                    for qt in range(NT):
                        ovp = ps.tile([128, 512], f32, tag="mm")
                        for i in range(KP):
                            nc.tensor.matmul(
                                ovp[:, :HD], et[:, 2 * i:2 * i + 2, qt * 128:(qt + 1) * 128],
                                vN8[:, 2 * i:2 * i + 2, h * HD:(h + 1) * HD],
                                start=(i == 0), stop=(i == KP - 1), perf_mode=DR)
                            # denominator column: same et weights, 0.25-vector.
                            # start=False even at i==0: the V-matmul's bank
                            # clear leaves has_written=0 here, so i==0
                            # overwrites rather than accumulates.
                            nc.tensor.matmul(
                                ovp[:, HD:HD + 1],
                                et[:, 2 * i:2 * i + 2, qt * 128:(qt + 1) * 128],
                                ones_sb[:],
                                start=False, stop=(i == KP - 1),
                                perf_mode=DR, skip_group_check=True)
                        rq = sm.tile([128, 1], f32, tag="rq")
                        nc.vector.reciprocal(rq[:], ovp[:, HD:HD + 1])
                        # ocat = (4*O) * 1/(s/4) = 16*o_norm
                        nc.vector.tensor_scalar(ocat[:, qt, h * HD:(h + 1) * HD],
                                                ovp[:, :HD], rq[:], None,
                                                op0=Alu.mult)
                otc16 = ws.tile([128, KH, CHUNK], fp16, tag="mixT_otc", bufs=2)
                for qt in range(NT):
                    nc.sync.dma_start_transpose(otc16[:, :, qt * 128:(qt + 1) * 128],
                                                ocat[:, qt, :])
                # convert per-qt column slice so wo(t=qt) starts as soon as
                # its transpose lands (pipelines the attention->wo boundary)
                otc8 = ws.tile([128, KH, CHUNK], fp8, tag="hidT_qT_otc8", bufs=2)
                for qt in range(NT):
                    eng = nc.gpsimd if qt % 2 == 0 else nc.vector
                    eng.tensor_copy(otc8[:, :, qt * 128:(qt + 1) * 128],
                                    otc16[:, :, qt * 128:(qt + 1) * 128])

                # ---------- stage 6: wo proj + residual + out LN ----------
                res = ws.tile([128, NT, H], fp16, tag="ocat_res")
                zT = ws.tile([128, KH, CHUNK], fp16, tag="kT_zT")
                mva6 = sm.tile([128, NT, 2], f32, tag="mva6")
                iva6 = sm.tile([128, NT], f32, tag="iva6")
                for t in range(NT):
                    for n in range(2):
                        ops_ = ps.tile([128, 512], f32, tag="mm")
                        for i in range(KP):
                            nc.tensor.matmul(
                                ops_[:],
                                otc8[:, 2 * i:2 * i + 2, t * 128:(t + 1) * 128],
                                wo_sb[:, 2 * i:2 * i + 2, n * 512:(n + 1) * 512],
                                start=(i == 0), stop=(i == KP - 1), perf_mode=DR)
                        # res = psum/(16*16) + mixN
                        nc.vector.scalar_tensor_tensor(
                            res[:, t, n * 512:(n + 1) * 512], ops_[:],
                            1.0 / (OSC * WSC),
                            mixN[:, t, n * 512:(n + 1) * 512],
                            op0=Alu.mult, op1=Alu.add)
                    st6 = sm.tile([128, 2, 6], f32, tag="st6b")
                    for half in range(2):
                        nc.vector.bn_stats(st6[:, half, :],
                                           res[:, t, half * 512:(half + 1) * 512])
                    nc.vector.bn_aggr(mva6[:, t, :], st6[:])
                    if t % 2 == 1:
                        g = t // 2
                        u2 = sm.tile([128, 2], f32, tag="u2")
                        nc.scalar.activation(u2[:], mva6[:, g * 2:(g + 1) * 2, 1:2],
                                             Act.Sqrt, bias=eps_sb[:])
                        nc.vector.reciprocal(iva6[:, g * 2:(g + 1) * 2], u2[:])
                        for tt in (t - 1, t):
                            zt = sm.tile([128, CHUNK], fp16, tag="zt")
                            nc.vector.tensor_scalar(zt[:], res[:, tt, :],
                                                    mva6[:, tt, 0:1],
                                                    iva6[:, tt:tt + 1],
                                                    op0=Alu.subtract, op1=Alu.mult)
                            nc.sync.dma_start_transpose(
                                zT[:, :, tt * 128:(tt + 1) * 128], zt[:])

                # ---------- stage 7: output projection ----------
                ych = ws.tile([128, NT, G], f32, tag="vN8_ych")
                yps = None
                for t in range(NT):
                    if t % 2 == 0:
                        yps = ps.tile([128, 2, G], f32, tag="mm")
                    for fi in range(KH):
                        nc.tensor.matmul(yps[:, t % 2, :], zT[:, fi, t * 128:(t + 1) * 128],
                                         gw_sb[:, fi, :],
                                         start=(fi == 0), stop=(fi == KH - 1))
                    if t % 2 == 1:
                        nc.vector.tensor_add(ych[:, t - 1:t + 1, :], yps[:], bw2_sb[:])
                    if t % 2 == 1:
                        q = t // 2
                        nc.sync.dma_start(
                            y.ap()[c, q * 256:(q + 1) * 256, :].rearrange(
                                "(t p) g -> p t g", p=128),
                            ych[:, t - 1:t + 1, :])

    nc.compile()
    return nc


def _get_compiled():
    global _COMPILED
    if _COMPILED is None:
        _COMPILED = _build()
    return _COMPILED


def _prep_inputs(inputs):
    f32 = np.float32

    def a(name):
        return np.asarray(inputs[name], dtype=f32)

    x = a("x")
    mw = a("mother_wavelets")
    scales = a("scales")
    norm = np.sqrt(np.sum(mw ** 2, axis=2, keepdims=True))
    kern = (mw / np.maximum(norm, 1e-12)) * (1.0 / (1.0 + np.exp(-scales)))
    kern = kern[0, :, :, 0]                      # (W, H)
    kernT = np.ascontiguousarray(kern.T).astype(F16)

    w1a = np.concatenate([a("mix_w1"), a("mix_b1")[None, :]], axis=0).astype(F16)
    gln = np.ascontiguousarray(a("mix_ln_g").reshape(KM, 128).T).astype(f32)
    bln = np.ascontiguousarray(a("mix_ln_b").reshape(KM, 128).T).astype(f32)
    w2 = a("mix_w2").astype(F16)
    b2c = np.ascontiguousarray(a("mix_b2").reshape(KH, 128).T).astype(f32)
    gw = (a("out_ln_g")[:, None] * a("out_w")).astype(F16)
    bw_vec = a("out_ln_b") @ a("out_w") + a("out_b")
    bw = np.tile(bw_vec[None, :], (128, 1)).astype(f32)

    shared = {
        "kernt": kernT, "w1a": w1a, "gln": gln, "bln": bln, "w2": w2,
        "b2c": b2c,
        "wq8": (a("wq") * WSC).astype(FP8), "wk8": (a("wk") * WSC).astype(FP8),
        "wv8": (a("wv") * WSC).astype(FP8), "wo8": (a("wo") * WSC).astype(FP8),
        "gw": gw, "bw": bw,
    }

    xc = x.reshape(N_CHUNKS, CHUNK, H)
    xt_all = np.ascontiguousarray(xc.transpose(0, 2, 1)).astype(F16)  # (16, H, CHUNK)
    in_maps = []
    for core in range(N_CORES):
        m = dict(shared)
        m["xt"] = np.ascontiguousarray(xt_all[core * CPC:(core + 1) * CPC])
        in_maps.append(m)
    return in_maps


def kernel(**inputs) -> np.ndarray:
    from concourse.bass_utils import run_bass_kernel_spmd

    nc = _get_compiled()
    in_maps = _prep_inputs(inputs)
    res = run_bass_kernel_spmd(nc, in_maps, core_ids=list(range(N_CORES)))
    out = np.concatenate([r["y"] for r in res.results], axis=0)  # (16, CHUNK, G)
    return out.reshape(B, S, G).astype(np.float32)
